# revision 46
# baseline (speedup 1.0000x reference)
"""Trainium2 Bass kernel for nn_CausalCrossAttention (B=4, S=2048, C=1024, NH=16, HD=64).

Sharding: DP over batch (4) x TP over heads (2 groups of 8), 8 NeuronCores.
Core c handles batch b = c // 2, head group g = c % 2 (heads 8g..8g+7).

Per-core algorithm (fp16 matmuls with fp32 PSUM accumulation; the final output
projection runs in float32r = tf32):
  - Host pre-transposes x/y to [C, S] and permutes weights head-pair-interleaved
    (pair p rows 128p..128p+127; even head partitions 0..63, odd head 64..127).
  - K-projection -> resident Kt [feat, S]; V-projection -> resident Vx
    [keys, feat] per 128-key tile.
  - Q-projection is done just-in-time per (q-window, pair) into a small tile.
  - Scores are computed transposed, S^T[k, q], two heads concurrently via PE
    row-groups (0,0)/(64,0) into ONE [128, 1024] two-bank PSUM tile; one
    exp(s/8) activation (strided 3D AP on diagonal tiles skips the dead
    columns) writes P^T (fp16). Causal: skip fully-masked k-tiles, shrink N
    for diagonal tiles, multiply diagonal 128-col blocks by a triangular mask.
  - PV: the two heads run COL-TILED concurrently (tile_position (0,0)/(0,64))
    into one [128, 512] PSUM bank: psum[0:64] += V_A^T P_A, psum[64:128] +=
    V_B^T P_B per k-tile.
  - Denominators: M=1 matmuls with the (mask) ones-column as lhsT, 4-way
    col-tiled so one 512-cycle pass covers both heads of TWO k-tiles,
    accumulated in a dedicated PSUM bank (partitions 0/32/64/96).
  - Normalize: two DVE adds combine the partial denominators into a [1,1024]
    row; two col-tiled K=1 broadcast matmuls (ones rows borrowed from the
    triangular-mask constant) spread it to 128 partitions; one reciprocal and
    one fused multiply write AOt [feat, S] (f32r, O-projection lhsT layout).
  - O-projection (interleaved after each q-window): out partial = AOt.T @ WoArr;
    host sums the two head-group partials and adds Wo_b.

DMA queues: all inputs stream on the sync queue in consumption order (wk/y0
interleaved first so the first K-proj matmul starts ~2us in); Wo plus all
output stores ride the gpsimd queue.
"""

import numpy as np
from contextlib import ExitStack

import concourse.bacc as bacc
import concourse.mybir as mybir
import concourse.tile as tile
from concourse import bass_utils

F32 = mybir.dt.float32
F16 = mybir.dt.float16
AF = mybir.ActivationFunctionType
OP = mybir.AluOpType

B, S, C, NH, HD = 4, 2048, 1024, 16, 64
N_CORES = 8



def build_program(s_len, cin, pairs, hd=64, has_mask=False, has_qkb=False, has_vb=False,
                  disable=frozenset()):
    """Build + compile the per-core Bass program. Returns the Bacc object."""
    assert s_len % 512 == 0 and cin % 128 == 0 and hd == 64
    n_sw = s_len // 512          # 512-wide s windows
    n_ct = cin // 128            # cin contraction tiles
    n_st = s_len // 128          # 128-wide s (key) tiles
    feat = 128 * pairs           # local feature dim (= 64 * local heads)
    n_h = 2 * pairs              # local heads
    cw_sizes = []
    rem = cin
    while rem > 0:
        cw_sizes.append(min(512, rem))
        rem -= 512

    nc = bacc.Bacc("TRN2", target_bir_lowering=False, debug=False,
                   num_devices=N_CORES)

    # all inputs host-folded to [128, *] with contiguous per-partition rows,
    # so every DMA is a single-row-block transfer (cheap descriptor gen)
    d_xT = nc.dram_tensor("xF", [128, n_sw * n_ct * 512], F16,
                          kind="ExternalInput")
    d_yT = nc.dram_tensor("yF", [128, n_sw * n_ct * 512], F16,
                          kind="ExternalInput")
    d_wq = nc.dram_tensor("wqF", [128, n_ct * feat], F16, kind="ExternalInput")
    d_wk = nc.dram_tensor("wkF", [128, n_ct * feat], F16, kind="ExternalInput")
    d_wv = nc.dram_tensor("wvF", [128, n_ct * feat], F16, kind="ExternalInput")
    d_wo = nc.dram_tensor("woF", [128, pairs * cin], F16, kind="ExternalInput")
    d_tri = nc.dram_tensor("tri", [128, 128], F32, kind="ExternalInput")
    d_mvc = nc.dram_tensor("mvc", [128, n_st], F16, kind="ExternalInput")
    d_sel4 = nc.dram_tensor("sel4", [97, 128], F16, kind="ExternalInput")
    if has_qkb:
        d_qb = nc.dram_tensor("qb", [128, pairs], F32, kind="ExternalInput")
        d_kb = nc.dram_tensor("kb", [128, pairs], F32, kind="ExternalInput")
    if has_vb:
        d_vbr = nc.dram_tensor("vbr", [128, feat], F32, kind="ExternalInput")
    if has_mask:
        d_mvst = nc.dram_tensor("mvst", [128, n_st], F32, kind="ExternalInput")
    d_out = nc.dram_tensor("out", [s_len, cin], F16, kind="ExternalOutput")

    with tile.TileContext(nc) as tc, ExitStack() as ctx:
        cpool = ctx.enter_context(tc.tile_pool(name="const", bufs=1))
        wpool = ctx.enter_context(tc.tile_pool(name="wts", bufs=3))
        bigp = ctx.enter_context(tc.tile_pool(name="big", bufs=1))
        sbuf = ctx.enter_context(tc.tile_pool(name="stream", bufs=5))
        qtwp = ctx.enter_context(tc.tile_pool(name="qtw", bufs=3))
        ppool = ctx.enter_context(tc.tile_pool(name="pt", bufs=7))
        rpool = ctx.enter_context(tc.tile_pool(name="rp", bufs=2))
        dpool = ctx.enter_context(tc.tile_pool(name="dp", bufs=2))
        # PSUM budget (8 banks): scores sA 2x[128,1024] = 4, proj ring
        # 2x[128,512] = 2, pvAB 1, psD 1.
        ps_pv = ctx.enter_context(tc.tile_pool(name="ps_pv", bufs=2, space="PSUM"))
        ps_s = ctx.enter_context(tc.tile_pool(name="ps_s", bufs=2, space="PSUM"))

        # --- PE clock warm-up ---------------------------------------------
        # The PE idles at 1.2 GHz until the activity monitor sees ~3.4us of
        # sustained work.  The DMA path only starts moving data ~8.7us in, so
        # run dummy matmuls on a memset tile (DVE boots much earlier) to have
        # the clock warm before the first real matmul.
        wtile = cpool.tile([128, 512], F16, tag="warm")
        nc.vector.memset(wtile[:], 0.0)
        wps = ps_s.tile([128, 1024], F32, tag="sA", name="warm")
        for _ in range(6):
            nc.tensor.matmul(wps[:, 0:512], wtile[:, 0:128], wtile[:],
                             start=True, stop=True)

        # --- input streams (host-folded layouts, two queues) ---------------
        wk = wpool.tile([128, n_ct * feat], F16, tag="w")
        wv = wpool.tile([128, n_ct * feat], F16, tag="w")
        wq = wpool.tile([128, n_ct * feat], F16, tag="w")
        wo = wpool.tile([128, pairs * cin], F16, tag="wo", bufs=1)

        def stream_tile(queue, dst, dsrc, col0, parts):
            # contiguous column-range copies (src layout == tile layout)
            for lo, hi in parts:
                queue.dma_start(dst[:, lo:hi], dsrc[:, col0 + lo:col0 + hi])

        yts_by_w = {}
        xts_by_w = {}
        wcols = n_ct * 512   # columns per s-window in the folded layout

        def emit_y_dmas(sw, parts=None):
            if sw in yts_by_w:
                return
            t = sbuf.tile([128, wcols], F16, tag="s")
            stream_tile(nc.sync, t, d_yT, sw * wcols, parts or [(0, wcols)])
            yts_by_w[sw] = t

        def emit_x_dmas(w, parts=None):
            if w in xts_by_w:
                return
            t = sbuf.tile([128, wcols], F16, tag="s")
            stream_tile(nc.sync, t, d_xT, w * wcols, parts or [(0, wcols)])
            xts_by_w[w] = t

        # weights ride the scalar queue (ScalarE is idle during startup and
        # the PL/gpsimd queue issues descriptors too slowly); activations the
        # sync queue.  Both stream in consumption order so the projection
        # chains can chase the transfers.
        q4 = [(i * wcols // 4, (i + 1) * wcols // 4) for i in range(4)]
        h0 = [(0, wcols // 2), (wcols // 2, wcols)]
        stream_tile(nc.scalar, wk, d_wk, 0, q4)
        emit_y_dmas(0, parts=q4)
        stream_tile(nc.scalar, wq, d_wq, 0, h0)
        emit_x_dmas(0, parts=h0)

        tri = cpool.tile([128, 128], F32, tag="tri")
        nc.sync.dma_start(tri[:], d_tri[:])
        mvc = cpool.tile([128, n_st], F16, tag="mvc")
        nc.sync.dma_start(mvc[:], d_mvc[:])
        sel4 = cpool.tile([97, 128], F16, tag="sel4")
        nc.sync.dma_start(sel4[:], d_sel4[:])
        if has_qkb:
            qb = cpool.tile([128, pairs], F32, tag="qb")
            nc.sync.dma_start(qb[:], d_qb[:])
            kb = cpool.tile([128, pairs], F32, tag="kb")
            nc.sync.dma_start(kb[:], d_kb[:])
        if has_vb:
            vbr = cpool.tile([128, feat], F32, tag="vbr")
            nc.sync.dma_start(vbr[:], d_vbr[:])
        if has_mask:
            mvst = cpool.tile([128, n_st], F32, tag="mvst")
            nc.sync.dma_start(mvst[:], d_mvst[:])

        stream_tile(nc.scalar, wv, d_wv, 0, [(0, n_ct * feat)])
        stream_tile(nc.scalar, wo, d_wo, 0, [(0, pairs * cin)])

        # big persistent tensors
        Kt = bigp.tile([128, pairs * s_len], F16, tag="kt")
        Vx = bigp.tile([128, n_st * feat], F16, tag="vx")
        AOt = bigp.tile([128, pairs * s_len], F16, tag="aot")

        def emit_k_mms(sw):
            yt = yts_by_w[sw]
            for mt in range(pairs):
                pk = ps_pv.tile([128, 512], F32, tag="proj")
                for ct in range(n_ct):
                    nc.tensor.matmul(
                        pk[:],
                        wk[:, ct * feat + mt * 128: ct * feat + (mt + 1) * 128],
                        yt[:, ct * 512:(ct + 1) * 512],
                        start=(ct == 0), stop=(ct == n_ct - 1))
                dst = Kt[:, mt * s_len + sw * 512: mt * s_len + (sw + 1) * 512]
                if has_qkb:
                    nc.vector.tensor_scalar_add(dst, pk[:], kb[:, mt:mt + 1])
                else:
                    nc.vector.tensor_copy(dst, pk[:])

        def emit_v_mms(sw):
            yt = yts_by_w[sw]
            for i in range(4):
                st = sw * 4 + i
                pvp = ps_pv.tile([128, feat], F32, tag="proj")
                for ct in range(n_ct):
                    nc.tensor.matmul(
                        pvp[:],
                        yt[:, ct * 512 + i * 128: ct * 512 + (i + 1) * 128],
                        wv[:, ct * feat:(ct + 1) * feat],
                        start=(ct == 0), stop=(ct == n_ct - 1))
                dst = Vx[:, st * feat:(st + 1) * feat]
                if has_vb:
                    nc.vector.scalar_tensor_tensor(
                        out=dst, in0=pvp[:], scalar=0.0, in1=vbr[:],
                        op0=OP.bypass, op1=OP.add)
                    if has_mask:
                        nc.vector.tensor_scalar_mul(dst, dst, mvst[:, st:st + 1])
                elif has_mask:
                    nc.vector.tensor_scalar_mul(dst, pvp[:], mvst[:, st:st + 1])
                else:
                    nc.vector.tensor_copy(dst, pvp[:])

        qtw_by_unit = {}

        def emit_qproj(w, p):
            xt = xts_by_w[w]
            # pq lives in the proj ring so the scores ring keeps both slots
            # free for the software-pipelined unit boundary
            pq = ps_pv.tile([128, 512], F32, tag="proj")
            for ct in range(n_ct):
                nc.tensor.matmul(
                    pq[:],
                    wq[:, ct * feat + p * 128: ct * feat + (p + 1) * 128],
                    xt[:, ct * 512:(ct + 1) * 512],
                    start=(ct == 0), stop=(ct == n_ct - 1))
            Qtw = qtwp.tile([128, 512], F16, tag="q")
            if has_qkb:
                nc.vector.tensor_scalar_add(Qtw[:], pq[:], qb[:, p:p + 1])
            else:
                # DVE copy: a ScalarE copy here queues behind the exp
                # backlog and stalls the next unit's first scores matmul
                nc.vector.tensor_copy(Qtw[:], pq[:])
            qtw_by_unit[(w, p)] = Qtw

        ot_by_st = {}

        def emit_oproj_part(st, cwi, p4_hi, po=None):
            co = sum(cw_sizes[:cwi])
            cw = cw_sizes[cwi]
            if po is None:
                po = ps_pv.tile([128, 512], F32, tag="proj")
            for p4 in range(p4_hi):
                nc.tensor.matmul(
                    po[:, 0:cw],
                    AOt[:, p4 * s_len + st * 128: p4 * s_len + (st + 1) * 128],
                    wo[:, p4 * cin + co: p4 * cin + co + cw],
                    start=(p4 == 0), stop=(p4 == pairs - 1))
            return po

        def emit_oproj_fin(st, cwi, po, p4_lo):
            co = sum(cw_sizes[:cwi])
            cw = cw_sizes[cwi]
            for p4 in range(p4_lo, pairs):
                nc.tensor.matmul(
                    po[:, 0:cw],
                    AOt[:, p4 * s_len + st * 128: p4 * s_len + (st + 1) * 128],
                    wo[:, p4 * cin + co: p4 * cin + co + cw],
                    start=False, stop=(p4 == pairs - 1))
            # stage all cw blocks of this st in one SBUF tile; single store
            if st in ot_by_st:
                ot = ot_by_st[st]
            else:
                ot = rpool.tile([128, cin], F16, tag="osb", bufs=3)
                ot_by_st[st] = ot
            nc.vector.tensor_copy(ot[:, co:co + cw], po[:, 0:cw])
            done = co + cw == cin
            if st >= n_st - 2:
                # final blocks: store each piece as soon as its copy lands,
                # spread across both DMA queues to shorten the drain
                q = nc.sync if (st + cwi) % 2 else nc.gpsimd
                q.dma_start(d_out[st * 128:(st + 1) * 128, co:co + cw],
                            ot[:, co:co + cw])
                if done:
                    del ot_by_st[st]
            elif done:
                del ot_by_st[st]
                q = nc.sync if st % 2 else nc.gpsimd
                q.dma_start(d_out[st * 128:(st + 1) * 128, :], ot[:])

        def emit_oproj_block(st):
            for cwi in range(len(cw_sizes)):
                po = emit_oproj_part(st, cwi, pairs)
                emit_oproj_fin(st, cwi, po, pairs)

        units = [(w, p) for w in range(n_sw) for p in range(pairs)]
        pre_pend = {}   # ui -> [(kt, c0, Pt)] score tiles pre-emitted by ui-1

        def emit_scores_kt(w, p, Qtw, kt):
            """Scores + exp (+ causal mask) for one 128-key tile."""
            k0 = kt * 128
            diag = kt >= 4 * w
            c0 = (kt - 4 * w) * 128 if diag else 0
            sAB = ps_s.tile([128, 1024], F32, tag="sA")
            nc.tensor.matmul(
                sAB[:, c0:512],
                Kt[0:64, p * s_len + k0: p * s_len + k0 + 128],
                Qtw[0:64, c0:512],
                start=True, stop=True, tile_position=(0, 0))
            nc.tensor.matmul(
                sAB[:, 512 + c0:1024],
                Kt[64:128, p * s_len + k0: p * s_len + k0 + 128],
                Qtw[64:128, c0:512],
                start=True, stop=True, tile_position=(64, 0))
            Pt = ppool.tile([128, 1024], F16, tag="Pt")
            if diag and c0 and "m3ap" not in disable:
                # one activation over both heads' live columns (3D AP
                # skips the dead [512:512+c0] region)
                nc.scalar.activation(
                    Pt.rearrange("p (h q) -> p h q", h=2)[:, :, c0:512],
                    sAB.rearrange("p (h q) -> p h q", h=2)[:, :, c0:512],
                    AF.Exp, scale=0.125)
            else:
                nc.scalar.activation(Pt[:, c0:1024], sAB[:, c0:1024],
                                     AF.Exp, scale=0.125)
            if diag:
                nc.vector.tensor_tensor(
                    out=Pt[:, c0:c0 + 128], in0=Pt[:, c0:c0 + 128],
                    in1=tri[:], op=OP.mult)
                nc.vector.tensor_tensor(
                    out=Pt[:, 512 + c0:512 + c0 + 128],
                    in0=Pt[:, 512 + c0:512 + c0 + 128],
                    in1=tri[:], op=OP.mult)
            return (kt, c0, Pt)

        def emit_unit(ui):
            w, p = units[ui]
            q0 = w * 512
            nxt = units[ui + 1] if ui + 1 < len(units) else None
            # O-projection blocks of the previous window, spread across units
            osts = []
            if w >= 1:
                lo, hi = p * 4 // pairs, (p + 1) * 4 // pairs
                osts = [4 * (w - 1) + i for i in range(lo, hi)]
            Qtw = qtw_by_unit.pop((w, p))
            nkt = 4 * (w + 1)
            n_pass = nkt // 2
            pvAB = ps_pv.tile([128, 512], F32, tag="pv", bufs=1)
            psD = ps_pv.tile([128, 512], F32, tag="psD", bufs=1)
            # Both banks hold multiple independent accumulation chains
            # (col-tiled regions), so no matmul may use the start=True bank
            # clear (measured: the whole-bank bit clear breaks the sibling
            # chain).  Zero-fill instead: accumulate-onto-0 and
            # overwrite-where-clear both yield the right value.
            nc.vector.memset(pvAB[:], 0.0)
            nc.vector.memset(psD[:], 0.0)
            # score tiles pre-emitted in the previous unit's tail
            pend = pre_pend.pop(ui, [])
            start_kt = len(pend)
            dready = []  # PV-emitted k-tiles awaiting a paired D-pass
            npass = [0]

            def emit_pv(pkt, pc0, pPt, stop):
                nc.tensor.matmul(
                    pvAB[0:64, pc0:512],
                    Vx[:, pkt * feat + (2 * p) * 64: pkt * feat + (2 * p) * 64 + 64],
                    pPt[:, pc0:512],
                    start=False, stop=stop, tile_position=(0, 0))
                nc.tensor.matmul(
                    pvAB[64:128, pc0:512],
                    Vx[:, pkt * feat + (2 * p + 1) * 64: pkt * feat + (2 * p + 1) * 64 + 64],
                    pPt[:, 512 + pc0:1024],
                    start=False, stop=stop, tile_position=(0, 64))
                dready.append((pkt, pc0, pPt))
                if len(dready) == 2:
                    emit_dpass(dready[0], dready[1])
                    dready.clear()

            def emit_dpass(d0, d1):
                last = npass[0] == n_pass - 1
                npass[0] += 1
                if "dpass" in disable:
                    return
                for gi, ((kt_, c0_, Pt_), half) in enumerate(
                        [(d0, 0), (d0, 1), (d1, 0), (d1, 1)]):
                    nc.tensor.matmul(
                        psD[32 * gi: 32 * gi + 1, c0_:512],
                        mvc[:, kt_:kt_ + 1],
                        Pt_[:, 512 * half + c0_: 512 * half + 512],
                        start=False,
                        stop=last, tile_position=(0, 32 * gi))

            for kt in range(start_kt, nkt):
                pend.append(emit_scores_kt(w, p, Qtw, kt))
                if kt == 3:
                    for st in osts:
                        emit_oproj_block(st)
                if len(pend) >= 3:
                    emit_pv(*pend.pop(0), stop=False)
            tail_pos = []
            if nxt is None:
                while pend:
                    emit_pv(*pend.pop(0), stop=(not pend))
                # tail O-proj: pair-0..2 partials for the first TWO st blocks
                # of the last window run before the final normalize chain
                # (pair 3's AOt isn't written yet); the second block borrows
                # a free scores-ring PSUM tile.
                st0 = 4 * (n_sw - 1)
                for cwi in range(len(cw_sizes)):
                    tail_pos.append(
                        (st0, cwi, emit_oproj_part(st0, cwi, pairs - 1)))
                po13 = ps_s.tile([128, 1024], F32, tag="sA", name="po13")
                for cwi in range(len(cw_sizes)):
                    tail_pos.append(
                        (st0 + 1, cwi,
                         emit_oproj_part(st0 + 1, cwi, pairs - 1,
                                         po=po13[:, cwi * 512:(cwi + 1) * 512])))
            else:
                # software-pipeline the unit boundary: Q-project the next
                # unit, then interleave its first two score tiles with this
                # unit's PV flush so the PE never waits on the exp backlog.
                emit_qproj(*nxt)
                Qtw_n = qtw_by_unit[nxt]
                wn, pn = nxt
                pend_n = [emit_scores_kt(wn, pn, Qtw_n, 0)]
                emit_pv(*pend.pop(0), stop=(not pend))
                pend_n.append(emit_scores_kt(wn, pn, Qtw_n, 1))
                while pend:
                    emit_pv(*pend.pop(0), stop=(not pend))
                pre_pend[ui + 1] = pend_n
            # normalize: one copy stages psD partitions 0..96 (the four D
            # partial rows live at 0/32/64/96; the rows between hold zeros
            # from the first unit's memset) into SBUF; a single K=97 matmul
            # against a selection constant then combines even/odd partials
            # and broadcasts them to all 128 partitions.
            dsb4 = dpool.tile([97, 512], F16, tag="dsb")
            if "dpass" in disable:
                nc.vector.memset(dsb4[:], 0.25)
            else:
                nc.vector.tensor_copy(dsb4[:], psD[0:97, 0:512])
            rsb = rpool.tile([128, 512], F32, tag="rsb")
            if "bcast" in disable:
                nc.vector.memset(rsb[:], 1.0)
            else:
                if nxt is None:
                    psBf = ps_s.tile([128, 1024], F32, tag="sA", name="psBf")
                    psB = psBf[:, 0:512]
                else:
                    psB = ps_pv.tile([128, 512], F32, tag="proj")
                # psB[pp, q] = D_A[q] for pp<64, D_B[q] for pp>=64
                nc.tensor.matmul(psB[:], sel4[:], dsb4[:],
                                 start=True, stop=True)
                nc.vector.reciprocal_approx_fast(out=rsb[:], in_=psB[:])
            if nxt is None:
                # split the final normalize multiply so the first tail
                # O-projection's weight load can start after 1/4 of it
                for qi in range(4):
                    nc.vector.tensor_tensor(
                        out=AOt[:, p * s_len + q0 + qi * 128:
                                p * s_len + q0 + (qi + 1) * 128],
                        in0=pvAB[:, qi * 128:(qi + 1) * 128],
                        in1=rsb[:, qi * 128:(qi + 1) * 128], op=OP.mult)
            else:
                nc.vector.tensor_tensor(
                    out=AOt[:, p * s_len + q0: p * s_len + q0 + 512],
                    in0=pvAB[:], in1=rsb[:], op=OP.mult)
            for st_, cwi_, po_ in tail_pos:
                emit_oproj_fin(st_, cwi_, po_, pairs - 1)

        # ---- emission schedule ----
        emit_k_mms(0)
        emit_qproj(0, 0)
        emit_v_mms(0)
        for ui in range(len(units)):
            w, p = units[ui]
            if p == 0 and w + 1 < n_sw:
                # prefetch next window's activations at window start
                emit_y_dmas(w + 1)
                emit_x_dmas(w + 1)
            emit_unit(ui)
            # defer window w+1's K/V projections into window w (the late,
            # exp-bound windows then carry more PE filler work)
            if w + 1 < n_sw:
                if p == 0:
                    emit_k_mms(w + 1)
                elif p == 1:
                    emit_v_mms(w + 1)
        for st in range(4 * (n_sw - 1) + 2, 4 * n_sw):   # last window's O-proj
            emit_oproj_block(st)

    nc.compile()
    return nc


_programs = {}


def _get_program(key):
    if key not in _programs:
        _programs[key] = build_program(S, C, 4, HD, *key)
    return _programs[key]


def make_core_inputs(x, y, mask, Wq_w, Wq_b, Wkv_w, Wkv_b, Wo_w,
                     s_len=S, cin=C, pairs=4, nh=NH):
    """Build the list of 8 per-core input dicts (host-side shard + permute)."""
    n_h = 2 * pairs
    feat = 128 * pairs
    has_mask = bool(np.any(mask))
    has_qkb = bool(np.any(Wq_b)) or bool(np.any(Wkv_b[:cin]))
    has_vb = bool(np.any(Wkv_b[cin:]))
    tri = np.triu(np.ones((128, 128), dtype=np.float32))
    n_st = s_len // 128

    in_maps = []
    for core in range(N_CORES):
        b = core // 2
        g = core % 2
        # feature permutation: col = 128*p + 64*half + d  <-  local head 2p+half
        cidx = np.arange(feat)
        pair_i = cidx // 128
        half = (cidx % 128) // 64
        d = cidx % 64
        qk_rows = (n_h * g + 2 * pair_i + half) * 64 + d
        v_rows = cin + (n_h * g + cidx // 64) * 64 + cidx % 64

        mvec = 1.0 - mask[b].astype(np.float32)
        sel4 = np.zeros((97, 128), dtype=np.float16)
        sel4[0, 0:64] = 1.0    # head-A even-k-tile partials
        sel4[32, 64:128] = 1.0  # head-B even
        sel4[64, 0:64] = 1.0   # head-A odd
        sel4[96, 64:128] = 1.0  # head-B odd
        n_ct = cin // 128
        n_sw = s_len // 512

        def fold(a):
            # [n*128, m] -> [128, n*m] with [p, i*m + j] = a[i*128+p, j]
            n = a.shape[0] // 128
            return np.ascontiguousarray(
                a.reshape(n, 128, a.shape[1]).transpose(1, 0, 2)
                .reshape(128, n * a.shape[1]))

        def fold_act(aT):
            # [cin, s_len] -> [128, n_sw * n_ct * 512] window-major:
            # [p, w*(n_ct*512) + ct*512 + j] = aT[ct*128+p, w*512+j]
            return np.ascontiguousarray(
                aT.reshape(n_ct, 128, n_sw, 512).transpose(1, 2, 0, 3)
                .reshape(128, n_sw * n_ct * 512))

        m = {
            "xF": fold_act(x[b].T.astype(np.float16)),
            "yF": fold_act(y[b].T.astype(np.float16)),
            "wqF": fold(Wq_w[qk_rows, :].T.astype(np.float16)),
            "wkF": fold(Wkv_w[qk_rows, :].T.astype(np.float16)),
            "wvF": fold(Wkv_w[v_rows, :].T.astype(np.float16)),
            "woF": fold(Wo_w[:, qk_rows].T.astype(np.float16)),
            "tri": tri,
            "mvc": np.ascontiguousarray(
                mvec.reshape(n_st, 128).T).astype(np.float16),
            "sel4": sel4,
        }
        if has_qkb:
            m["qb"] = np.ascontiguousarray(
                Wq_b[qk_rows].reshape(pairs, 128).T)
            m["kb"] = np.ascontiguousarray(
                Wkv_b[qk_rows].reshape(pairs, 128).T)
        if has_vb:
            m["vbr"] = np.tile(Wkv_b[v_rows][None, :], (128, 1))
        if has_mask:
            m["mvst"] = np.ascontiguousarray(
                mvec.reshape(n_st, 128).T)
        in_maps.append(m)
    return in_maps, (has_mask, has_qkb, has_vb)


def run(x, y, mask, Wq_w, Wq_b, Wkv_w, Wkv_b, Wo_w, Wo_b, trace=False):
    x = np.asarray(x, dtype=np.float32)
    y = np.asarray(y, dtype=np.float32)
    mask = np.asarray(mask)
    Wq_w = np.asarray(Wq_w, dtype=np.float32)
    Wq_b = np.asarray(Wq_b, dtype=np.float32)
    Wkv_w = np.asarray(Wkv_w, dtype=np.float32)
    Wkv_b = np.asarray(Wkv_b, dtype=np.float32)
    Wo_w = np.asarray(Wo_w, dtype=np.float32)
    Wo_b = np.asarray(Wo_b, dtype=np.float32)

    in_maps, flags = make_core_inputs(x, y, mask, Wq_w, Wq_b, Wkv_w, Wkv_b, Wo_w)
    nc = _get_program(flags)
    res = bass_utils.run_bass_kernel_spmd(
        nc, in_maps, core_ids=list(range(N_CORES)), trace=trace)
    out = np.empty((B, S, C), dtype=np.float32)
    for b in range(B):
        out[b] = (res.results[2 * b]["out"].astype(np.float32)
                  + res.results[2 * b + 1]["out"].astype(np.float32) + Wo_b)
    return out, res


def kernel(x, y, mask, Wq_w, Wq_b, Wkv_w, Wkv_b, Wo_w, Wo_b):
    out, _ = run(x, y, mask, Wq_w, Wq_b, Wkv_w, Wkv_b, Wo_w, Wo_b, trace=False)
    return out



# revision 47
# speedup vs baseline: 1.1878x; 1.1878x over previous
"""Trainium2 Bass kernel for nn_CausalCrossAttention (B=4, S=2048, C=1024, NH=16, HD=64).

Sharding: DP over batch (4) x TP over heads (2 groups of 8), 8 NeuronCores.
Core c handles batch b = c // 2, head group g = c % 2 (heads 8g..8g+7).

Per-core algorithm (fp16 matmuls with fp32 PSUM accumulation; the final output
projection runs in float32r = tf32):
  - Host pre-transposes x/y to [C, S] and permutes weights head-pair-interleaved
    (pair p rows 128p..128p+127; even head partitions 0..63, odd head 64..127).
  - K-projection -> resident Kt [feat, S]; V-projection -> resident Vx
    [keys, feat] per 128-key tile.
  - Q-projection is done just-in-time per (q-window, pair) into a small tile.
  - Scores are computed transposed, S^T[k, q], two heads concurrently via PE
    row-groups (0,0)/(64,0) into ONE [128, 1024] two-bank PSUM tile; one
    exp(s/8) activation (strided 3D AP on diagonal tiles skips the dead
    columns) writes P^T (fp16). Causal: skip fully-masked k-tiles, shrink N
    for diagonal tiles, multiply diagonal 128-col blocks by a triangular mask.
  - PV: the two heads run COL-TILED concurrently (tile_position (0,0)/(0,64))
    into one [128, 512] PSUM bank: psum[0:64] += V_A^T P_A, psum[64:128] +=
    V_B^T P_B per k-tile.
  - Denominators: M=1 matmuls with the (mask) ones-column as lhsT, 4-way
    col-tiled so one 512-cycle pass covers both heads of TWO k-tiles,
    accumulated in a dedicated PSUM bank (partitions 0/32/64/96).
  - Normalize: two DVE adds combine the partial denominators into a [1,1024]
    row; two col-tiled K=1 broadcast matmuls (ones rows borrowed from the
    triangular-mask constant) spread it to 128 partitions; one reciprocal and
    one fused multiply write AOt [feat, S] (f32r, O-projection lhsT layout).
  - O-projection (interleaved after each q-window): out partial = AOt.T @ WoArr;
    host sums the two head-group partials and adds Wo_b.

DMA queues: all inputs stream on the sync queue in consumption order (wk/y0
interleaved first so the first K-proj matmul starts ~2us in); Wo plus all
output stores ride the gpsimd queue.
"""

import numpy as np
from contextlib import ExitStack

import concourse.bacc as bacc
import concourse.mybir as mybir
import concourse.tile as tile
from concourse import bass_utils

F32 = mybir.dt.float32
F16 = mybir.dt.float16
AF = mybir.ActivationFunctionType
OP = mybir.AluOpType

B, S, C, NH, HD = 4, 2048, 1024, 16, 64
N_CORES = 8



def build_program(s_len, cin, pairs, hd=64, has_mask=False, has_qkb=False, has_vb=False,
                  disable=frozenset()):
    """Build + compile the per-core Bass program. Returns the Bacc object."""
    assert s_len % 512 == 0 and cin % 128 == 0 and hd == 64
    n_sw = s_len // 512          # 512-wide s windows
    n_ct = cin // 128            # cin contraction tiles
    n_st = s_len // 128          # 128-wide s (key) tiles
    feat = 128 * pairs           # local feature dim (= 64 * local heads)
    n_h = 2 * pairs              # local heads
    cw_sizes = []
    rem = cin
    while rem > 0:
        cw_sizes.append(min(512, rem))
        rem -= 512

    nc = bacc.Bacc("TRN2", target_bir_lowering=False, debug=False,
                   num_devices=N_CORES)

    # all inputs host-folded to [128, *] with contiguous per-partition rows,
    # so every DMA is a single-row-block transfer (cheap descriptor gen)
    d_xT = nc.dram_tensor("xF", [128, n_sw * n_ct * 512], F16,
                          kind="ExternalInput")
    d_yT = nc.dram_tensor("yF", [128, n_sw * n_ct * 512], F16,
                          kind="ExternalInput")
    d_wq = nc.dram_tensor("wqF", [128, n_ct * feat], F16, kind="ExternalInput")
    d_wk = nc.dram_tensor("wkF", [128, n_ct * feat], F16, kind="ExternalInput")
    d_wv = nc.dram_tensor("wvF", [128, n_ct * feat], F16, kind="ExternalInput")
    d_wo = nc.dram_tensor("woF", [128, pairs * cin], F16, kind="ExternalInput")
    d_tri = nc.dram_tensor("tri", [128, 128], F32, kind="ExternalInput")
    d_mvc = nc.dram_tensor("mvc", [128, n_st], F16, kind="ExternalInput")
    d_sel4 = nc.dram_tensor("sel4", [97, 128], F16, kind="ExternalInput")
    if has_qkb:
        d_qb = nc.dram_tensor("qb", [128, pairs], F32, kind="ExternalInput")
        d_kb = nc.dram_tensor("kb", [128, pairs], F32, kind="ExternalInput")
    if has_vb:
        d_vbr = nc.dram_tensor("vbr", [128, feat], F32, kind="ExternalInput")
    if has_mask:
        d_mvst = nc.dram_tensor("mvst", [128, n_st], F32, kind="ExternalInput")
    d_out = nc.dram_tensor("out", [s_len, cin], F16, kind="ExternalOutput")

    with tile.TileContext(nc) as tc, ExitStack() as ctx:
        cpool = ctx.enter_context(tc.tile_pool(name="const", bufs=1))
        wpool = ctx.enter_context(tc.tile_pool(name="wts", bufs=3))
        bigp = ctx.enter_context(tc.tile_pool(name="big", bufs=1))
        sbuf = ctx.enter_context(tc.tile_pool(name="stream", bufs=5))
        qtwp = ctx.enter_context(tc.tile_pool(name="qtw", bufs=3))
        ppool = ctx.enter_context(tc.tile_pool(name="pt", bufs=5))
        rpool = ctx.enter_context(tc.tile_pool(name="rp", bufs=2))
        dpool = ctx.enter_context(tc.tile_pool(name="dp", bufs=2))
        # PSUM budget (8 banks): scores sA 2x[128,1024] = 4, proj ring
        # 2x[128,512] = 2, pvAB 1, psD 1.
        ps_pv = ctx.enter_context(tc.tile_pool(name="ps_pv", bufs=2, space="PSUM"))
        ps_s = ctx.enter_context(tc.tile_pool(name="ps_s", bufs=2, space="PSUM"))

        # --- PE clock warm-up ---------------------------------------------
        # The PE idles at 1.2 GHz until the activity monitor sees ~3.4us of
        # sustained work.  The DMA path only starts moving data ~8.7us in, so
        # run dummy matmuls on a memset tile (DVE boots much earlier) to have
        # the clock warm before the first real matmul.
        wtile = cpool.tile([128, 512], F16, tag="warm")
        nc.vector.memset(wtile[:], 0.0)
        wps = ps_s.tile([128, 1024], F32, tag="sA", name="warm")
        for _ in range(6):
            nc.tensor.matmul(wps[:, 0:512], wtile[:, 0:128], wtile[:],
                             start=True, stop=True)

        # --- input streams (host-folded layouts, two queues) ---------------
        wk = wpool.tile([128, n_ct * feat], F16, tag="w")
        wv = wpool.tile([128, n_ct * feat], F16, tag="w")
        wq = wpool.tile([128, n_ct * feat], F16, tag="w")
        wo = wpool.tile([128, pairs * cin], F16, tag="wo", bufs=1)

        def stream_tile(queue, dst, dsrc, col0, parts):
            # contiguous column-range copies (src layout == tile layout)
            for lo, hi in parts:
                queue.dma_start(dst[:, lo:hi], dsrc[:, col0 + lo:col0 + hi])

        yts_by_w = {}
        xts_by_w = {}
        wcols = n_ct * 512   # columns per s-window in the folded layout

        def emit_y_dmas(sw, parts=None):
            if sw in yts_by_w:
                return
            t = sbuf.tile([128, wcols], F16, tag="s")
            stream_tile(nc.sync, t, d_yT, sw * wcols, parts or [(0, wcols)])
            yts_by_w[sw] = t

        def emit_x_dmas(w, parts=None):
            if w in xts_by_w:
                return
            t = sbuf.tile([128, wcols], F16, tag="s")
            stream_tile(nc.sync, t, d_xT, w * wcols, parts or [(0, wcols)])
            xts_by_w[w] = t

        # weights ride the scalar queue (ScalarE is idle during startup and
        # the PL/gpsimd queue issues descriptors too slowly); activations the
        # sync queue.  Both stream in consumption order so the projection
        # chains can chase the transfers.
        q4 = [(i * wcols // 4, (i + 1) * wcols // 4) for i in range(4)]
        h0 = [(0, wcols // 2), (wcols // 2, wcols)]
        stream_tile(nc.scalar, wk, d_wk, 0, q4)
        emit_y_dmas(0, parts=q4)
        stream_tile(nc.scalar, wq, d_wq, 0, h0)
        emit_x_dmas(0, parts=h0)

        tri = cpool.tile([128, 128], F32, tag="tri")
        nc.sync.dma_start(tri[:], d_tri[:])
        mvc = cpool.tile([128, n_st], F16, tag="mvc")
        nc.sync.dma_start(mvc[:], d_mvc[:])
        sel4 = cpool.tile([97, 128], F16, tag="sel4")
        nc.sync.dma_start(sel4[:], d_sel4[:])
        if has_qkb:
            qb = cpool.tile([128, pairs], F32, tag="qb")
            nc.sync.dma_start(qb[:], d_qb[:])
            kb = cpool.tile([128, pairs], F32, tag="kb")
            nc.sync.dma_start(kb[:], d_kb[:])
        if has_vb:
            vbr = cpool.tile([128, feat], F32, tag="vbr")
            nc.sync.dma_start(vbr[:], d_vbr[:])
        if has_mask:
            mvst = cpool.tile([128, n_st], F32, tag="mvst")
            nc.sync.dma_start(mvst[:], d_mvst[:])

        stream_tile(nc.scalar, wv, d_wv, 0, [(0, n_ct * feat)])
        stream_tile(nc.scalar, wo, d_wo, 0, [(0, pairs * cin)])

        # big persistent tensors
        Kt = bigp.tile([128, pairs * s_len], F16, tag="kt")
        Vx = bigp.tile([128, n_st * feat], F16, tag="vx")
        AOt = bigp.tile([128, pairs * s_len], F16, tag="aot")

        def emit_k_mms(sw):
            yt = yts_by_w[sw]
            for mt in range(pairs):
                pk = ps_pv.tile([128, 512], F32, tag="proj")
                for ct in range(n_ct):
                    nc.tensor.matmul(
                        pk[:],
                        wk[:, ct * feat + mt * 128: ct * feat + (mt + 1) * 128],
                        yt[:, ct * 512:(ct + 1) * 512],
                        start=(ct == 0), stop=(ct == n_ct - 1))
                dst = Kt[:, mt * s_len + sw * 512: mt * s_len + (sw + 1) * 512]
                if has_qkb:
                    nc.vector.tensor_scalar_add(dst, pk[:], kb[:, mt:mt + 1])
                else:
                    nc.vector.tensor_copy(dst, pk[:])

        def emit_v_mms(sw):
            yt = yts_by_w[sw]
            for i in range(4):
                st = sw * 4 + i
                pvp = ps_pv.tile([128, feat], F32, tag="proj")
                for ct in range(n_ct):
                    nc.tensor.matmul(
                        pvp[:],
                        yt[:, ct * 512 + i * 128: ct * 512 + (i + 1) * 128],
                        wv[:, ct * feat:(ct + 1) * feat],
                        start=(ct == 0), stop=(ct == n_ct - 1))
                dst = Vx[:, st * feat:(st + 1) * feat]
                if has_vb:
                    nc.vector.scalar_tensor_tensor(
                        out=dst, in0=pvp[:], scalar=0.0, in1=vbr[:],
                        op0=OP.bypass, op1=OP.add)
                    if has_mask:
                        nc.vector.tensor_scalar_mul(dst, dst, mvst[:, st:st + 1])
                elif has_mask:
                    nc.vector.tensor_scalar_mul(dst, pvp[:], mvst[:, st:st + 1])
                else:
                    nc.vector.tensor_copy(dst, pvp[:])

        qtw_by_unit = {}

        def emit_qproj(w, p):
            xt = xts_by_w[w]
            # pq lives in the proj ring so the scores ring keeps both slots
            # free for the software-pipelined unit boundary
            pq = ps_pv.tile([128, 512], F32, tag="proj")
            for ct in range(n_ct):
                nc.tensor.matmul(
                    pq[:],
                    wq[:, ct * feat + p * 128: ct * feat + (p + 1) * 128],
                    xt[:, ct * 512:(ct + 1) * 512],
                    start=(ct == 0), stop=(ct == n_ct - 1))
            Qtw = qtwp.tile([128, 512], F16, tag="q")
            if has_qkb:
                nc.vector.tensor_scalar_add(Qtw[:], pq[:], qb[:, p:p + 1])
            else:
                # DVE copy: a ScalarE copy here queues behind the exp
                # backlog and stalls the next unit's first scores matmul
                nc.vector.tensor_copy(Qtw[:], pq[:])
            qtw_by_unit[(w, p)] = Qtw

        ot_by_st = {}

        def emit_oproj_part(st, cwi, p4_hi, po=None):
            co = sum(cw_sizes[:cwi])
            cw = cw_sizes[cwi]
            if po is None:
                po = ps_pv.tile([128, 512], F32, tag="proj")
            for p4 in range(p4_hi):
                nc.tensor.matmul(
                    po[:, 0:cw],
                    AOt[:, p4 * s_len + st * 128: p4 * s_len + (st + 1) * 128],
                    wo[:, p4 * cin + co: p4 * cin + co + cw],
                    start=(p4 == 0), stop=(p4 == pairs - 1))
            return po

        def emit_oproj_fin(st, cwi, po, p4_lo):
            co = sum(cw_sizes[:cwi])
            cw = cw_sizes[cwi]
            for p4 in range(p4_lo, pairs):
                nc.tensor.matmul(
                    po[:, 0:cw],
                    AOt[:, p4 * s_len + st * 128: p4 * s_len + (st + 1) * 128],
                    wo[:, p4 * cin + co: p4 * cin + co + cw],
                    start=False, stop=(p4 == pairs - 1))
            # stage all cw blocks of this st in one SBUF tile; single store
            if st in ot_by_st:
                ot = ot_by_st[st]
            else:
                ot = rpool.tile([128, cin], F16, tag="osb", bufs=3)
                ot_by_st[st] = ot
            nc.vector.tensor_copy(ot[:, co:co + cw], po[:, 0:cw])
            done = co + cw == cin
            if st >= n_st - 2:
                # final blocks: store each piece as soon as its copy lands,
                # spread across both DMA queues to shorten the drain
                q = nc.sync if (st + cwi) % 2 else nc.gpsimd
                q.dma_start(d_out[st * 128:(st + 1) * 128, co:co + cw],
                            ot[:, co:co + cw])
                if done:
                    del ot_by_st[st]
            elif done:
                del ot_by_st[st]
                q = nc.sync if st % 2 else nc.gpsimd
                q.dma_start(d_out[st * 128:(st + 1) * 128, :], ot[:])

        def emit_oproj_block(st):
            for cwi in range(len(cw_sizes)):
                po = emit_oproj_part(st, cwi, pairs)
                emit_oproj_fin(st, cwi, po, pairs)

        units = [(w, p) for w in range(n_sw) for p in range(pairs)]
        pre_pend = {}   # ui -> [(kt, c0, Pt)] score tiles pre-emitted by ui-1

        def emit_scores_kt(w, p, Qtw, kt):
            """Scores + exp (+ causal mask) for one 128-key tile."""
            k0 = kt * 128
            diag = kt >= 4 * w
            c0 = (kt - 4 * w) * 128 if diag else 0
            sAB = ps_s.tile([128, 1024], F32, tag="sA")
            nc.tensor.matmul(
                sAB[:, c0:512],
                Kt[0:64, p * s_len + k0: p * s_len + k0 + 128],
                Qtw[0:64, c0:512],
                start=True, stop=True, tile_position=(0, 0))
            nc.tensor.matmul(
                sAB[:, 512 + c0:1024],
                Kt[64:128, p * s_len + k0: p * s_len + k0 + 128],
                Qtw[64:128, c0:512],
                start=True, stop=True, tile_position=(64, 0))
            Pt = ppool.tile([128, 1024], F16, tag="Pt")
            if diag and c0 and "m3ap" not in disable:
                # one activation over both heads' live columns (3D AP
                # skips the dead [512:512+c0] region)
                nc.scalar.activation(
                    Pt.rearrange("p (h q) -> p h q", h=2)[:, :, c0:512],
                    sAB.rearrange("p (h q) -> p h q", h=2)[:, :, c0:512],
                    AF.Exp, scale=0.125)
            else:
                nc.scalar.activation(Pt[:, c0:1024], sAB[:, c0:1024],
                                     AF.Exp, scale=0.125)
            if diag:
                nc.vector.tensor_tensor(
                    out=Pt[:, c0:c0 + 128], in0=Pt[:, c0:c0 + 128],
                    in1=tri[:], op=OP.mult)
                nc.vector.tensor_tensor(
                    out=Pt[:, 512 + c0:512 + c0 + 128],
                    in0=Pt[:, 512 + c0:512 + c0 + 128],
                    in1=tri[:], op=OP.mult)
            return (kt, c0, Pt)

        def emit_unit(ui):
            w, p = units[ui]
            q0 = w * 512
            nxt = units[ui + 1] if ui + 1 < len(units) else None
            # O-projection blocks of the previous window, spread across units
            osts = []
            if w >= 1:
                lo, hi = p * 4 // pairs, (p + 1) * 4 // pairs
                osts = [4 * (w - 1) + i for i in range(lo, hi)]
            Qtw = qtw_by_unit.pop((w, p))
            nkt = 4 * (w + 1)
            n_pass = nkt // 2
            pvAB = ps_pv.tile([128, 512], F32, tag="pv", bufs=1)
            psD = ps_pv.tile([128, 512], F32, tag="psD", bufs=1)
            # Both banks hold multiple independent accumulation chains
            # (col-tiled regions), so no matmul may use the start=True bank
            # clear (measured: the whole-bank bit clear breaks the sibling
            # chain).  Zero-fill instead: accumulate-onto-0 and
            # overwrite-where-clear both yield the right value.
            nc.vector.memset(pvAB[:], 0.0)
            nc.vector.memset(psD[:], 0.0)
            # score tiles pre-emitted in the previous unit's tail
            pend = pre_pend.pop(ui, [])
            start_kt = len(pend)
            dready = []  # PV-emitted k-tiles awaiting a paired D-pass
            npass = [0]

            def emit_pv(pkt, pc0, pPt, stop):
                nc.tensor.matmul(
                    pvAB[0:64, pc0:512],
                    Vx[:, pkt * feat + (2 * p) * 64: pkt * feat + (2 * p) * 64 + 64],
                    pPt[:, pc0:512],
                    start=False, stop=stop, tile_position=(0, 0))
                nc.tensor.matmul(
                    pvAB[64:128, pc0:512],
                    Vx[:, pkt * feat + (2 * p + 1) * 64: pkt * feat + (2 * p + 1) * 64 + 64],
                    pPt[:, 512 + pc0:1024],
                    start=False, stop=stop, tile_position=(0, 64))
                dready.append((pkt, pc0, pPt))
                if len(dready) == 2:
                    emit_dpass(dready[0], dready[1])
                    dready.clear()

            def emit_dpass(d0, d1):
                last = npass[0] == n_pass - 1
                npass[0] += 1
                if "dpass" in disable:
                    return
                for gi, ((kt_, c0_, Pt_), half) in enumerate(
                        [(d0, 0), (d0, 1), (d1, 0), (d1, 1)]):
                    nc.tensor.matmul(
                        psD[32 * gi: 32 * gi + 1, c0_:512],
                        mvc[:, kt_:kt_ + 1],
                        Pt_[:, 512 * half + c0_: 512 * half + 512],
                        start=False,
                        stop=last, tile_position=(0, 32 * gi))

            for kt in range(start_kt, nkt):
                pend.append(emit_scores_kt(w, p, Qtw, kt))
                if kt == 3:
                    for st in osts:
                        emit_oproj_block(st)
                if len(pend) >= 3:
                    emit_pv(*pend.pop(0), stop=False)
            tail_pos = []
            if nxt is None:
                while pend:
                    emit_pv(*pend.pop(0), stop=(not pend))
                # tail O-proj: pair-0..2 partials for the first TWO st blocks
                # of the last window run before the final normalize chain
                # (pair 3's AOt isn't written yet); the second block borrows
                # a free scores-ring PSUM tile.
                st0 = 4 * (n_sw - 1)
                for cwi in range(len(cw_sizes)):
                    tail_pos.append(
                        (st0, cwi, emit_oproj_part(st0, cwi, pairs - 1)))
                po13 = ps_s.tile([128, 1024], F32, tag="sA", name="po13")
                for cwi in range(len(cw_sizes)):
                    tail_pos.append(
                        (st0 + 1, cwi,
                         emit_oproj_part(st0 + 1, cwi, pairs - 1,
                                         po=po13[:, cwi * 512:(cwi + 1) * 512])))
            else:
                # software-pipeline the unit boundary: Q-project the next
                # unit, then interleave its first two score tiles with this
                # unit's PV flush so the PE never waits on the exp backlog.
                emit_qproj(*nxt)
                Qtw_n = qtw_by_unit[nxt]
                wn, pn = nxt
                pend_n = [emit_scores_kt(wn, pn, Qtw_n, 0)]
                emit_pv(*pend.pop(0), stop=(not pend))
                pend_n.append(emit_scores_kt(wn, pn, Qtw_n, 1))
                while pend:
                    emit_pv(*pend.pop(0), stop=(not pend))
                pre_pend[ui + 1] = pend_n
            # normalize: one copy stages psD partitions 0..96 (the four D
            # partial rows live at 0/32/64/96; the rows between hold zeros
            # from the first unit's memset) into SBUF; a single K=97 matmul
            # against a selection constant then combines even/odd partials
            # and broadcasts them to all 128 partitions.
            dsb4 = dpool.tile([97, 512], F16, tag="dsb")
            if "dpass" in disable:
                nc.vector.memset(dsb4[:], 0.25)
            else:
                nc.vector.tensor_copy(dsb4[:], psD[0:97, 0:512])
            rsb = rpool.tile([128, 512], F32, tag="rsb")
            if "bcast" in disable:
                nc.vector.memset(rsb[:], 1.0)
            else:
                if nxt is None:
                    psBf = ps_s.tile([128, 1024], F32, tag="sA", name="psBf")
                    psB = psBf[:, 0:512]
                else:
                    psB = ps_pv.tile([128, 512], F32, tag="proj")
                # psB[pp, q] = D_A[q] for pp<64, D_B[q] for pp>=64
                nc.tensor.matmul(psB[:], sel4[:], dsb4[:],
                                 start=True, stop=True)
                nc.vector.reciprocal_approx_fast(out=rsb[:], in_=psB[:])
            if nxt is None:
                # split the final normalize multiply so the first tail
                # O-projection's weight load can start after 1/4 of it
                for qi in range(4):
                    nc.vector.tensor_tensor(
                        out=AOt[:, p * s_len + q0 + qi * 128:
                                p * s_len + q0 + (qi + 1) * 128],
                        in0=pvAB[:, qi * 128:(qi + 1) * 128],
                        in1=rsb[:, qi * 128:(qi + 1) * 128], op=OP.mult)
            else:
                nc.vector.tensor_tensor(
                    out=AOt[:, p * s_len + q0: p * s_len + q0 + 512],
                    in0=pvAB[:], in1=rsb[:], op=OP.mult)
            for st_, cwi_, po_ in tail_pos:
                emit_oproj_fin(st_, cwi_, po_, pairs - 1)

        # ---- emission schedule ----
        emit_k_mms(0)
        emit_qproj(0, 0)
        emit_v_mms(0)
        for ui in range(len(units)):
            w, p = units[ui]
            if p == 0 and w + 1 < n_sw:
                # prefetch next window's activations at window start
                emit_y_dmas(w + 1)
                emit_x_dmas(w + 1)
            emit_unit(ui)
            # defer window w+1's K/V projections into window w (the late,
            # exp-bound windows then carry more PE filler work)
            if w + 1 < n_sw:
                if p == 0:
                    emit_k_mms(w + 1)
                elif p == 1:
                    emit_v_mms(w + 1)
        for st in range(4 * (n_sw - 1) + 2, 4 * n_sw):   # last window's O-proj
            emit_oproj_block(st)

    nc.compile()
    return nc


_programs = {}


def _get_program(key):
    if key not in _programs:
        _programs[key] = build_program(S, C, 4, HD, *key)
    return _programs[key]


def make_core_inputs(x, y, mask, Wq_w, Wq_b, Wkv_w, Wkv_b, Wo_w,
                     s_len=S, cin=C, pairs=4, nh=NH):
    """Build the list of 8 per-core input dicts (host-side shard + permute)."""
    n_h = 2 * pairs
    feat = 128 * pairs
    has_mask = bool(np.any(mask))
    has_qkb = bool(np.any(Wq_b)) or bool(np.any(Wkv_b[:cin]))
    has_vb = bool(np.any(Wkv_b[cin:]))
    tri = np.triu(np.ones((128, 128), dtype=np.float32))
    n_st = s_len // 128

    in_maps = []
    for core in range(N_CORES):
        b = core // 2
        g = core % 2
        # feature permutation: col = 128*p + 64*half + d  <-  local head 2p+half
        cidx = np.arange(feat)
        pair_i = cidx // 128
        half = (cidx % 128) // 64
        d = cidx % 64
        qk_rows = (n_h * g + 2 * pair_i + half) * 64 + d
        v_rows = cin + (n_h * g + cidx // 64) * 64 + cidx % 64

        mvec = 1.0 - mask[b].astype(np.float32)
        sel4 = np.zeros((97, 128), dtype=np.float16)
        sel4[0, 0:64] = 1.0    # head-A even-k-tile partials
        sel4[32, 64:128] = 1.0  # head-B even
        sel4[64, 0:64] = 1.0   # head-A odd
        sel4[96, 64:128] = 1.0  # head-B odd
        n_ct = cin // 128
        n_sw = s_len // 512

        def fold(a):
            # [n*128, m] -> [128, n*m] with [p, i*m + j] = a[i*128+p, j]
            n = a.shape[0] // 128
            return np.ascontiguousarray(
                a.reshape(n, 128, a.shape[1]).transpose(1, 0, 2)
                .reshape(128, n * a.shape[1]))

        def fold_act(aT):
            # [cin, s_len] -> [128, n_sw * n_ct * 512] window-major:
            # [p, w*(n_ct*512) + ct*512 + j] = aT[ct*128+p, w*512+j]
            return np.ascontiguousarray(
                aT.reshape(n_ct, 128, n_sw, 512).transpose(1, 2, 0, 3)
                .reshape(128, n_sw * n_ct * 512))

        m = {
            "xF": fold_act(x[b].T.astype(np.float16)),
            "yF": fold_act(y[b].T.astype(np.float16)),
            "wqF": fold(Wq_w[qk_rows, :].T.astype(np.float16)),
            "wkF": fold(Wkv_w[qk_rows, :].T.astype(np.float16)),
            "wvF": fold(Wkv_w[v_rows, :].T.astype(np.float16)),
            "woF": fold(Wo_w[:, qk_rows].T.astype(np.float16)),
            "tri": tri,
            "mvc": np.ascontiguousarray(
                mvec.reshape(n_st, 128).T).astype(np.float16),
            "sel4": sel4,
        }
        if has_qkb:
            m["qb"] = np.ascontiguousarray(
                Wq_b[qk_rows].reshape(pairs, 128).T)
            m["kb"] = np.ascontiguousarray(
                Wkv_b[qk_rows].reshape(pairs, 128).T)
        if has_vb:
            m["vbr"] = np.tile(Wkv_b[v_rows][None, :], (128, 1))
        if has_mask:
            m["mvst"] = np.ascontiguousarray(
                mvec.reshape(n_st, 128).T)
        in_maps.append(m)
    return in_maps, (has_mask, has_qkb, has_vb)


def run(x, y, mask, Wq_w, Wq_b, Wkv_w, Wkv_b, Wo_w, Wo_b, trace=False):
    x = np.asarray(x, dtype=np.float32)
    y = np.asarray(y, dtype=np.float32)
    mask = np.asarray(mask)
    Wq_w = np.asarray(Wq_w, dtype=np.float32)
    Wq_b = np.asarray(Wq_b, dtype=np.float32)
    Wkv_w = np.asarray(Wkv_w, dtype=np.float32)
    Wkv_b = np.asarray(Wkv_b, dtype=np.float32)
    Wo_w = np.asarray(Wo_w, dtype=np.float32)
    Wo_b = np.asarray(Wo_b, dtype=np.float32)

    in_maps, flags = make_core_inputs(x, y, mask, Wq_w, Wq_b, Wkv_w, Wkv_b, Wo_w)
    nc = _get_program(flags)
    res = bass_utils.run_bass_kernel_spmd(
        nc, in_maps, core_ids=list(range(N_CORES)), trace=trace)
    out = np.empty((B, S, C), dtype=np.float32)
    for b in range(B):
        out[b] = (res.results[2 * b]["out"].astype(np.float32)
                  + res.results[2 * b + 1]["out"].astype(np.float32) + Wo_b)
    return out, res


def kernel(x, y, mask, Wq_w, Wq_b, Wkv_w, Wkv_b, Wo_w, Wo_b):
    out, _ = run(x, y, mask, Wq_w, Wq_b, Wkv_w, Wkv_b, Wo_w, Wo_b, trace=False)
    return out



# revision 48
# speedup vs baseline: 1.1889x; 1.0009x over previous
"""Trainium2 Bass kernel for nn_CausalCrossAttention (B=4, S=2048, C=1024, NH=16, HD=64).

Sharding: DP over batch (4) x TP over heads (2 groups of 8), 8 NeuronCores.
Core c handles batch b = c // 2, head group g = c % 2 (heads 8g..8g+7).

Per-core algorithm (fp16 matmuls everywhere with fp32 PSUM accumulation):
  - Host folds every input to a [128, *] layout with contiguous
    per-partition rows, so each DMA is a single cheap row-block transfer
    (descriptor generation, not bandwidth, limits the startup otherwise);
    weights stream on the scalar queue, activations on the sync queue, both
    in consumption order.  Dummy matmuls on a memset tile warm the PE clock
    (1.2 -> 2.4 GHz) during the ~9us DMA boot window.
  - K-projection -> resident Kt [feat, S]; V-projection -> resident Vx
    [keys, feat] per 128-key tile; Q-projection just-in-time per
    (q-window, pair) via the proj PSUM ring.
  - Scores are computed transposed, S^T[k, q], two heads via PE row-groups
    (0,0)/(64,0) into ONE [128, 1024] two-bank PSUM tile; one exp(s/8)
    activation (strided 3D AP on diagonal tiles skips the dead columns)
    writes P^T (fp16). Causal: skip fully-masked k-tiles, shrink N for
    diagonal tiles, multiply diagonal 128-col blocks by a triangular mask.
  - PV: the two heads run col-tiled (tile_position (0,0)/(0,64)) into one
    [128, 512] PSUM bank, interleaved 2-3 tiles behind the scores so the
    exp pipeline stays covered.
  - Denominators: M=1 matmuls with the (mask) ones-column as lhsT, 4-way
    col-tiled so one pass covers both heads of TWO k-tiles, accumulated in
    a dedicated PSUM bank (partitions 0/32/64/96).
  - Normalize: ONE strided DVE copy stages psD partitions 0..96 in SBUF;
    a single K=97 matmul against a selection constant combines even/odd
    partials AND broadcasts D_A/D_B to the 128 partitions; reciprocal +
    fused multiply write AOt [feat, S] (fp16, O-projection lhsT layout).
  - Unit boundaries are software-pipelined: each unit Q-projects the next
    unit and pre-emits its first two score tiles interleaved with its own
    PV flush, so the PE never waits on the exp backlog at a boundary.
  - O-projection (interleaved after each q-window): out partial
    (AOt.T @ Wo, fp16) staged per 128-row block in one SBUF tile, stored
    fp16 with one DMA (split across both queues for the final two blocks);
    host sums the two head-group partials in fp32 and adds Wo_b.  In the
    last unit, partials for the first TWO trailing blocks run before the
    final normalize (the second borrows a free scores-ring PSUM tile).

PSUM hygiene: pvAB/psD hold multiple independent accumulation chains, and
start=True clears has_written for the WHOLE bank, so both are zero-filled
via DVE memset and all chains accumulate with start=False (measured: the
start=True trick corrupts the sibling chain).  Keep the Pt pool at 5 bufs
(7 measured ~55us slower - SBUF placement is sensitive).
"""

import numpy as np
from contextlib import ExitStack

import concourse.bacc as bacc
import concourse.mybir as mybir
import concourse.tile as tile
from concourse import bass_utils

F32 = mybir.dt.float32
F16 = mybir.dt.float16
AF = mybir.ActivationFunctionType
OP = mybir.AluOpType

B, S, C, NH, HD = 4, 2048, 1024, 16, 64
N_CORES = 8



def build_program(s_len, cin, pairs, hd=64, has_mask=False, has_qkb=False, has_vb=False,
                  disable=frozenset()):
    """Build + compile the per-core Bass program. Returns the Bacc object."""
    assert s_len % 512 == 0 and cin % 128 == 0 and hd == 64
    n_sw = s_len // 512          # 512-wide s windows
    n_ct = cin // 128            # cin contraction tiles
    n_st = s_len // 128          # 128-wide s (key) tiles
    feat = 128 * pairs           # local feature dim (= 64 * local heads)
    n_h = 2 * pairs              # local heads
    cw_sizes = []
    rem = cin
    while rem > 0:
        cw_sizes.append(min(512, rem))
        rem -= 512

    nc = bacc.Bacc("TRN2", target_bir_lowering=False, debug=False,
                   num_devices=N_CORES)

    # all inputs host-folded to [128, *] with contiguous per-partition rows,
    # so every DMA is a single-row-block transfer (cheap descriptor gen)
    d_xT = nc.dram_tensor("xF", [128, n_sw * n_ct * 512], F16,
                          kind="ExternalInput")
    d_yT = nc.dram_tensor("yF", [128, n_sw * n_ct * 512], F16,
                          kind="ExternalInput")
    d_wq = nc.dram_tensor("wqF", [128, n_ct * feat], F16, kind="ExternalInput")
    d_wk = nc.dram_tensor("wkF", [128, n_ct * feat], F16, kind="ExternalInput")
    d_wv = nc.dram_tensor("wvF", [128, n_ct * feat], F16, kind="ExternalInput")
    d_wo = nc.dram_tensor("woF", [128, pairs * cin], F16, kind="ExternalInput")
    d_tri = nc.dram_tensor("tri", [128, 128], F32, kind="ExternalInput")
    d_mvc = nc.dram_tensor("mvc", [128, n_st], F16, kind="ExternalInput")
    d_sel4 = nc.dram_tensor("sel4", [97, 128], F16, kind="ExternalInput")
    if has_qkb:
        d_qb = nc.dram_tensor("qb", [128, pairs], F32, kind="ExternalInput")
        d_kb = nc.dram_tensor("kb", [128, pairs], F32, kind="ExternalInput")
    if has_vb:
        d_vbr = nc.dram_tensor("vbr", [128, feat], F32, kind="ExternalInput")
    if has_mask:
        d_mvst = nc.dram_tensor("mvst", [128, n_st], F32, kind="ExternalInput")
    d_out = nc.dram_tensor("out", [s_len, cin], F16, kind="ExternalOutput")

    with tile.TileContext(nc) as tc, ExitStack() as ctx:
        cpool = ctx.enter_context(tc.tile_pool(name="const", bufs=1))
        wpool = ctx.enter_context(tc.tile_pool(name="wts", bufs=3))
        bigp = ctx.enter_context(tc.tile_pool(name="big", bufs=1))
        sbuf = ctx.enter_context(tc.tile_pool(name="stream", bufs=5))
        qtwp = ctx.enter_context(tc.tile_pool(name="qtw", bufs=3))
        ppool = ctx.enter_context(tc.tile_pool(name="pt", bufs=5))
        rpool = ctx.enter_context(tc.tile_pool(name="rp", bufs=2))
        dpool = ctx.enter_context(tc.tile_pool(name="dp", bufs=2))
        # PSUM budget (8 banks): scores sA 2x[128,1024] = 4, proj ring
        # 2x[128,512] = 2, pvAB 1, psD 1.
        ps_pv = ctx.enter_context(tc.tile_pool(name="ps_pv", bufs=2, space="PSUM"))
        ps_s = ctx.enter_context(tc.tile_pool(name="ps_s", bufs=2, space="PSUM"))

        # --- PE clock warm-up ---------------------------------------------
        # The PE idles at 1.2 GHz until the activity monitor sees ~3.4us of
        # sustained work.  The DMA path only starts moving data ~8.7us in, so
        # run dummy matmuls on a memset tile (DVE boots much earlier) to have
        # the clock warm before the first real matmul.
        wtile = cpool.tile([128, 512], F16, tag="warm")
        nc.vector.memset(wtile[:], 0.0)
        wps = ps_s.tile([128, 1024], F32, tag="sA", name="warm")
        for _ in range(6):
            nc.tensor.matmul(wps[:, 0:512], wtile[:, 0:128], wtile[:],
                             start=True, stop=True)

        # --- input streams (host-folded layouts, two queues) ---------------
        wk = wpool.tile([128, n_ct * feat], F16, tag="w")
        wv = wpool.tile([128, n_ct * feat], F16, tag="w")
        wq = wpool.tile([128, n_ct * feat], F16, tag="w")
        wo = wpool.tile([128, pairs * cin], F16, tag="wo", bufs=1)

        def stream_tile(queue, dst, dsrc, col0, parts):
            # contiguous column-range copies (src layout == tile layout)
            for lo, hi in parts:
                queue.dma_start(dst[:, lo:hi], dsrc[:, col0 + lo:col0 + hi])

        yts_by_w = {}
        xts_by_w = {}
        wcols = n_ct * 512   # columns per s-window in the folded layout

        def emit_y_dmas(sw, parts=None):
            if sw in yts_by_w:
                return
            t = sbuf.tile([128, wcols], F16, tag="s")
            stream_tile(nc.sync, t, d_yT, sw * wcols, parts or [(0, wcols)])
            yts_by_w[sw] = t

        def emit_x_dmas(w, parts=None):
            if w in xts_by_w:
                return
            t = sbuf.tile([128, wcols], F16, tag="s")
            stream_tile(nc.sync, t, d_xT, w * wcols, parts or [(0, wcols)])
            xts_by_w[w] = t

        # weights ride the scalar queue (ScalarE is idle during startup and
        # the PL/gpsimd queue issues descriptors too slowly); activations the
        # sync queue.  Both stream in consumption order so the projection
        # chains can chase the transfers.
        q4 = [(i * wcols // 4, (i + 1) * wcols // 4) for i in range(4)]
        h0 = [(0, wcols // 2), (wcols // 2, wcols)]
        stream_tile(nc.scalar, wk, d_wk, 0, q4)
        emit_y_dmas(0, parts=q4)
        stream_tile(nc.scalar, wq, d_wq, 0, h0)
        emit_x_dmas(0, parts=h0)

        tri = cpool.tile([128, 128], F32, tag="tri")
        nc.sync.dma_start(tri[:], d_tri[:])
        mvc = cpool.tile([128, n_st], F16, tag="mvc")
        nc.sync.dma_start(mvc[:], d_mvc[:])
        sel4 = cpool.tile([97, 128], F16, tag="sel4")
        nc.sync.dma_start(sel4[:], d_sel4[:])
        if has_qkb:
            qb = cpool.tile([128, pairs], F32, tag="qb")
            nc.sync.dma_start(qb[:], d_qb[:])
            kb = cpool.tile([128, pairs], F32, tag="kb")
            nc.sync.dma_start(kb[:], d_kb[:])
        if has_vb:
            vbr = cpool.tile([128, feat], F32, tag="vbr")
            nc.sync.dma_start(vbr[:], d_vbr[:])
        if has_mask:
            mvst = cpool.tile([128, n_st], F32, tag="mvst")
            nc.sync.dma_start(mvst[:], d_mvst[:])

        stream_tile(nc.scalar, wv, d_wv, 0, [(0, n_ct * feat)])
        stream_tile(nc.scalar, wo, d_wo, 0, [(0, pairs * cin)])

        # big persistent tensors
        Kt = bigp.tile([128, pairs * s_len], F16, tag="kt")
        Vx = bigp.tile([128, n_st * feat], F16, tag="vx")
        AOt = bigp.tile([128, pairs * s_len], F16, tag="aot")

        def emit_k_mms(sw):
            yt = yts_by_w[sw]
            for mt in range(pairs):
                pk = ps_pv.tile([128, 512], F32, tag="proj")
                for ct in range(n_ct):
                    nc.tensor.matmul(
                        pk[:],
                        wk[:, ct * feat + mt * 128: ct * feat + (mt + 1) * 128],
                        yt[:, ct * 512:(ct + 1) * 512],
                        start=(ct == 0), stop=(ct == n_ct - 1))
                dst = Kt[:, mt * s_len + sw * 512: mt * s_len + (sw + 1) * 512]
                if has_qkb:
                    nc.vector.tensor_scalar_add(dst, pk[:], kb[:, mt:mt + 1])
                else:
                    nc.vector.tensor_copy(dst, pk[:])

        def emit_v_mms(sw):
            yt = yts_by_w[sw]
            for i in range(4):
                st = sw * 4 + i
                pvp = ps_pv.tile([128, feat], F32, tag="proj")
                for ct in range(n_ct):
                    nc.tensor.matmul(
                        pvp[:],
                        yt[:, ct * 512 + i * 128: ct * 512 + (i + 1) * 128],
                        wv[:, ct * feat:(ct + 1) * feat],
                        start=(ct == 0), stop=(ct == n_ct - 1))
                dst = Vx[:, st * feat:(st + 1) * feat]
                if has_vb:
                    nc.vector.scalar_tensor_tensor(
                        out=dst, in0=pvp[:], scalar=0.0, in1=vbr[:],
                        op0=OP.bypass, op1=OP.add)
                    if has_mask:
                        nc.vector.tensor_scalar_mul(dst, dst, mvst[:, st:st + 1])
                elif has_mask:
                    nc.vector.tensor_scalar_mul(dst, pvp[:], mvst[:, st:st + 1])
                else:
                    nc.vector.tensor_copy(dst, pvp[:])

        qtw_by_unit = {}

        def emit_qproj(w, p):
            xt = xts_by_w[w]
            # pq lives in the proj ring so the scores ring keeps both slots
            # free for the software-pipelined unit boundary
            pq = ps_pv.tile([128, 512], F32, tag="proj")
            for ct in range(n_ct):
                nc.tensor.matmul(
                    pq[:],
                    wq[:, ct * feat + p * 128: ct * feat + (p + 1) * 128],
                    xt[:, ct * 512:(ct + 1) * 512],
                    start=(ct == 0), stop=(ct == n_ct - 1))
            Qtw = qtwp.tile([128, 512], F16, tag="q")
            if has_qkb:
                nc.vector.tensor_scalar_add(Qtw[:], pq[:], qb[:, p:p + 1])
            else:
                # DVE copy: a ScalarE copy here queues behind the exp
                # backlog and stalls the next unit's first scores matmul
                nc.vector.tensor_copy(Qtw[:], pq[:])
            qtw_by_unit[(w, p)] = Qtw

        ot_by_st = {}

        def emit_oproj_part(st, cwi, p4_hi, po=None):
            co = sum(cw_sizes[:cwi])
            cw = cw_sizes[cwi]
            if po is None:
                po = ps_pv.tile([128, 512], F32, tag="proj")
            for p4 in range(p4_hi):
                nc.tensor.matmul(
                    po[:, 0:cw],
                    AOt[:, p4 * s_len + st * 128: p4 * s_len + (st + 1) * 128],
                    wo[:, p4 * cin + co: p4 * cin + co + cw],
                    start=(p4 == 0), stop=(p4 == pairs - 1))
            return po

        def emit_oproj_fin(st, cwi, po, p4_lo):
            co = sum(cw_sizes[:cwi])
            cw = cw_sizes[cwi]
            for p4 in range(p4_lo, pairs):
                nc.tensor.matmul(
                    po[:, 0:cw],
                    AOt[:, p4 * s_len + st * 128: p4 * s_len + (st + 1) * 128],
                    wo[:, p4 * cin + co: p4 * cin + co + cw],
                    start=False, stop=(p4 == pairs - 1))
            # stage all cw blocks of this st in one SBUF tile; single store
            if st in ot_by_st:
                ot = ot_by_st[st]
            else:
                ot = rpool.tile([128, cin], F16, tag="osb", bufs=3)
                ot_by_st[st] = ot
            nc.vector.tensor_copy(ot[:, co:co + cw], po[:, 0:cw])
            done = co + cw == cin
            if st >= n_st - 2:
                # final blocks: store each piece as soon as its copy lands,
                # spread across both DMA queues to shorten the drain
                q = nc.sync if (st + cwi) % 2 else nc.gpsimd
                q.dma_start(d_out[st * 128:(st + 1) * 128, co:co + cw],
                            ot[:, co:co + cw])
                if done:
                    del ot_by_st[st]
            elif done:
                del ot_by_st[st]
                q = nc.sync if st % 2 else nc.gpsimd
                q.dma_start(d_out[st * 128:(st + 1) * 128, :], ot[:])

        def emit_oproj_block(st):
            for cwi in range(len(cw_sizes)):
                po = emit_oproj_part(st, cwi, pairs)
                emit_oproj_fin(st, cwi, po, pairs)

        units = [(w, p) for w in range(n_sw) for p in range(pairs)]
        pre_pend = {}   # ui -> [(kt, c0, Pt)] score tiles pre-emitted by ui-1

        def emit_scores_kt(w, p, Qtw, kt):
            """Scores + exp (+ causal mask) for one 128-key tile."""
            k0 = kt * 128
            diag = kt >= 4 * w
            c0 = (kt - 4 * w) * 128 if diag else 0
            sAB = ps_s.tile([128, 1024], F32, tag="sA")
            nc.tensor.matmul(
                sAB[:, c0:512],
                Kt[0:64, p * s_len + k0: p * s_len + k0 + 128],
                Qtw[0:64, c0:512],
                start=True, stop=True, tile_position=(0, 0))
            nc.tensor.matmul(
                sAB[:, 512 + c0:1024],
                Kt[64:128, p * s_len + k0: p * s_len + k0 + 128],
                Qtw[64:128, c0:512],
                start=True, stop=True, tile_position=(64, 0))
            Pt = ppool.tile([128, 1024], F16, tag="Pt")
            if diag and c0 and "m3ap" not in disable:
                # one activation over both heads' live columns (3D AP
                # skips the dead [512:512+c0] region)
                nc.scalar.activation(
                    Pt.rearrange("p (h q) -> p h q", h=2)[:, :, c0:512],
                    sAB.rearrange("p (h q) -> p h q", h=2)[:, :, c0:512],
                    AF.Exp, scale=0.125)
            else:
                nc.scalar.activation(Pt[:, c0:1024], sAB[:, c0:1024],
                                     AF.Exp, scale=0.125)
            if diag:
                nc.vector.tensor_tensor(
                    out=Pt[:, c0:c0 + 128], in0=Pt[:, c0:c0 + 128],
                    in1=tri[:], op=OP.mult)
                nc.vector.tensor_tensor(
                    out=Pt[:, 512 + c0:512 + c0 + 128],
                    in0=Pt[:, 512 + c0:512 + c0 + 128],
                    in1=tri[:], op=OP.mult)
            return (kt, c0, Pt)

        def emit_unit(ui):
            w, p = units[ui]
            q0 = w * 512
            nxt = units[ui + 1] if ui + 1 < len(units) else None
            # O-projection blocks of the previous window, spread across units
            osts = []
            if w >= 1:
                lo, hi = p * 4 // pairs, (p + 1) * 4 // pairs
                osts = [4 * (w - 1) + i for i in range(lo, hi)]
            Qtw = qtw_by_unit.pop((w, p))
            nkt = 4 * (w + 1)
            n_pass = nkt // 2
            pvAB = ps_pv.tile([128, 512], F32, tag="pv", bufs=1)
            psD = ps_pv.tile([128, 512], F32, tag="psD", bufs=1)
            # Both banks hold multiple independent accumulation chains
            # (col-tiled regions), so no matmul may use the start=True bank
            # clear (measured: the whole-bank bit clear breaks the sibling
            # chain).  Zero-fill instead: accumulate-onto-0 and
            # overwrite-where-clear both yield the right value.
            nc.vector.memset(pvAB[:], 0.0)
            nc.vector.memset(psD[:], 0.0)
            # score tiles pre-emitted in the previous unit's tail
            pend = pre_pend.pop(ui, [])
            start_kt = len(pend)
            dready = []  # PV-emitted k-tiles awaiting a paired D-pass
            npass = [0]

            def emit_pv(pkt, pc0, pPt, stop):
                nc.tensor.matmul(
                    pvAB[0:64, pc0:512],
                    Vx[:, pkt * feat + (2 * p) * 64: pkt * feat + (2 * p) * 64 + 64],
                    pPt[:, pc0:512],
                    start=False, stop=stop, tile_position=(0, 0))
                nc.tensor.matmul(
                    pvAB[64:128, pc0:512],
                    Vx[:, pkt * feat + (2 * p + 1) * 64: pkt * feat + (2 * p + 1) * 64 + 64],
                    pPt[:, 512 + pc0:1024],
                    start=False, stop=stop, tile_position=(0, 64))
                dready.append((pkt, pc0, pPt))
                if len(dready) == 2:
                    emit_dpass(dready[0], dready[1])
                    dready.clear()

            def emit_dpass(d0, d1):
                last = npass[0] == n_pass - 1
                npass[0] += 1
                if "dpass" in disable:
                    return
                for gi, ((kt_, c0_, Pt_), half) in enumerate(
                        [(d0, 0), (d0, 1), (d1, 0), (d1, 1)]):
                    nc.tensor.matmul(
                        psD[32 * gi: 32 * gi + 1, c0_:512],
                        mvc[:, kt_:kt_ + 1],
                        Pt_[:, 512 * half + c0_: 512 * half + 512],
                        start=False,
                        stop=last, tile_position=(0, 32 * gi))

            for kt in range(start_kt, nkt):
                pend.append(emit_scores_kt(w, p, Qtw, kt))
                if kt == 3:
                    for st in osts:
                        emit_oproj_block(st)
                if len(pend) >= 3:
                    emit_pv(*pend.pop(0), stop=False)
            tail_pos = []
            if nxt is None:
                while pend:
                    emit_pv(*pend.pop(0), stop=(not pend))
                # tail O-proj: pair-0..2 partials for the first TWO st blocks
                # of the last window run before the final normalize chain
                # (pair 3's AOt isn't written yet); the second block borrows
                # a free scores-ring PSUM tile.
                st0 = 4 * (n_sw - 1)
                for cwi in range(len(cw_sizes)):
                    tail_pos.append(
                        (st0, cwi, emit_oproj_part(st0, cwi, pairs - 1)))
                po13 = ps_s.tile([128, 1024], F32, tag="sA", name="po13")
                for cwi in range(len(cw_sizes)):
                    tail_pos.append(
                        (st0 + 1, cwi,
                         emit_oproj_part(st0 + 1, cwi, pairs - 1,
                                         po=po13[:, cwi * 512:(cwi + 1) * 512])))
            else:
                # software-pipeline the unit boundary: Q-project the next
                # unit, then interleave its first two score tiles with this
                # unit's PV flush so the PE never waits on the exp backlog.
                emit_qproj(*nxt)
                Qtw_n = qtw_by_unit[nxt]
                wn, pn = nxt
                pend_n = [emit_scores_kt(wn, pn, Qtw_n, 0)]
                emit_pv(*pend.pop(0), stop=(not pend))
                pend_n.append(emit_scores_kt(wn, pn, Qtw_n, 1))
                while pend:
                    emit_pv(*pend.pop(0), stop=(not pend))
                pre_pend[ui + 1] = pend_n
            # normalize: one copy stages psD partitions 0..96 (the four D
            # partial rows live at 0/32/64/96; the rows between hold zeros
            # from the first unit's memset) into SBUF; a single K=97 matmul
            # against a selection constant then combines even/odd partials
            # and broadcasts them to all 128 partitions.
            dsb4 = dpool.tile([97, 512], F16, tag="dsb")
            if "dpass" in disable:
                nc.vector.memset(dsb4[:], 0.25)
            else:
                nc.vector.tensor_copy(dsb4[:], psD[0:97, 0:512])
            rsb = rpool.tile([128, 512], F32, tag="rsb")
            if "bcast" in disable:
                nc.vector.memset(rsb[:], 1.0)
            else:
                if nxt is None:
                    psBf = ps_s.tile([128, 1024], F32, tag="sA", name="psBf")
                    psB = psBf[:, 0:512]
                else:
                    psB = ps_pv.tile([128, 512], F32, tag="proj")
                # psB[pp, q] = D_A[q] for pp<64, D_B[q] for pp>=64
                nc.tensor.matmul(psB[:], sel4[:], dsb4[:],
                                 start=True, stop=True)
                nc.vector.reciprocal_approx_fast(out=rsb[:], in_=psB[:])
            if nxt is None:
                # split the final normalize multiply so the first tail
                # O-projection's weight load can start after 1/4 of it
                for qi in range(4):
                    nc.vector.tensor_tensor(
                        out=AOt[:, p * s_len + q0 + qi * 128:
                                p * s_len + q0 + (qi + 1) * 128],
                        in0=pvAB[:, qi * 128:(qi + 1) * 128],
                        in1=rsb[:, qi * 128:(qi + 1) * 128], op=OP.mult)
            else:
                nc.vector.tensor_tensor(
                    out=AOt[:, p * s_len + q0: p * s_len + q0 + 512],
                    in0=pvAB[:], in1=rsb[:], op=OP.mult)
            for st_, cwi_, po_ in tail_pos:
                emit_oproj_fin(st_, cwi_, po_, pairs - 1)

        # ---- emission schedule ----
        emit_k_mms(0)
        emit_qproj(0, 0)
        emit_v_mms(0)
        for ui in range(len(units)):
            w, p = units[ui]
            if p == 0 and w + 1 < n_sw:
                # prefetch next window's activations at window start
                emit_y_dmas(w + 1)
                emit_x_dmas(w + 1)
            emit_unit(ui)
            # defer window w+1's K/V projections into window w (the late,
            # exp-bound windows then carry more PE filler work)
            if w + 1 < n_sw:
                if p == 0:
                    emit_k_mms(w + 1)
                elif p == 1:
                    emit_v_mms(w + 1)
        for st in range(4 * (n_sw - 1) + 2, 4 * n_sw):   # last window's O-proj
            emit_oproj_block(st)

    nc.compile()
    return nc


_programs = {}


def _get_program(key):
    if key not in _programs:
        _programs[key] = build_program(S, C, 4, HD, *key)
    return _programs[key]


def make_core_inputs(x, y, mask, Wq_w, Wq_b, Wkv_w, Wkv_b, Wo_w,
                     s_len=S, cin=C, pairs=4, nh=NH):
    """Build the list of 8 per-core input dicts (host-side shard + permute)."""
    n_h = 2 * pairs
    feat = 128 * pairs
    has_mask = bool(np.any(mask))
    has_qkb = bool(np.any(Wq_b)) or bool(np.any(Wkv_b[:cin]))
    has_vb = bool(np.any(Wkv_b[cin:]))
    tri = np.triu(np.ones((128, 128), dtype=np.float32))
    n_st = s_len // 128

    in_maps = []
    for core in range(N_CORES):
        b = core // 2
        g = core % 2
        # feature permutation: col = 128*p + 64*half + d  <-  local head 2p+half
        cidx = np.arange(feat)
        pair_i = cidx // 128
        half = (cidx % 128) // 64
        d = cidx % 64
        qk_rows = (n_h * g + 2 * pair_i + half) * 64 + d
        v_rows = cin + (n_h * g + cidx // 64) * 64 + cidx % 64

        mvec = 1.0 - mask[b].astype(np.float32)
        sel4 = np.zeros((97, 128), dtype=np.float16)
        sel4[0, 0:64] = 1.0    # head-A even-k-tile partials
        sel4[32, 64:128] = 1.0  # head-B even
        sel4[64, 0:64] = 1.0   # head-A odd
        sel4[96, 64:128] = 1.0  # head-B odd
        n_ct = cin // 128
        n_sw = s_len // 512

        def fold(a):
            # [n*128, m] -> [128, n*m] with [p, i*m + j] = a[i*128+p, j]
            n = a.shape[0] // 128
            return np.ascontiguousarray(
                a.reshape(n, 128, a.shape[1]).transpose(1, 0, 2)
                .reshape(128, n * a.shape[1]))

        def fold_act(aT):
            # [cin, s_len] -> [128, n_sw * n_ct * 512] window-major:
            # [p, w*(n_ct*512) + ct*512 + j] = aT[ct*128+p, w*512+j]
            return np.ascontiguousarray(
                aT.reshape(n_ct, 128, n_sw, 512).transpose(1, 2, 0, 3)
                .reshape(128, n_sw * n_ct * 512))

        m = {
            "xF": fold_act(x[b].T.astype(np.float16)),
            "yF": fold_act(y[b].T.astype(np.float16)),
            "wqF": fold(Wq_w[qk_rows, :].T.astype(np.float16)),
            "wkF": fold(Wkv_w[qk_rows, :].T.astype(np.float16)),
            "wvF": fold(Wkv_w[v_rows, :].T.astype(np.float16)),
            "woF": fold(Wo_w[:, qk_rows].T.astype(np.float16)),
            "tri": tri,
            "mvc": np.ascontiguousarray(
                mvec.reshape(n_st, 128).T).astype(np.float16),
            "sel4": sel4,
        }
        if has_qkb:
            m["qb"] = np.ascontiguousarray(
                Wq_b[qk_rows].reshape(pairs, 128).T)
            m["kb"] = np.ascontiguousarray(
                Wkv_b[qk_rows].reshape(pairs, 128).T)
        if has_vb:
            m["vbr"] = np.tile(Wkv_b[v_rows][None, :], (128, 1))
        if has_mask:
            m["mvst"] = np.ascontiguousarray(
                mvec.reshape(n_st, 128).T)
        in_maps.append(m)
    return in_maps, (has_mask, has_qkb, has_vb)


def run(x, y, mask, Wq_w, Wq_b, Wkv_w, Wkv_b, Wo_w, Wo_b, trace=False):
    x = np.asarray(x, dtype=np.float32)
    y = np.asarray(y, dtype=np.float32)
    mask = np.asarray(mask)
    Wq_w = np.asarray(Wq_w, dtype=np.float32)
    Wq_b = np.asarray(Wq_b, dtype=np.float32)
    Wkv_w = np.asarray(Wkv_w, dtype=np.float32)
    Wkv_b = np.asarray(Wkv_b, dtype=np.float32)
    Wo_w = np.asarray(Wo_w, dtype=np.float32)
    Wo_b = np.asarray(Wo_b, dtype=np.float32)

    in_maps, flags = make_core_inputs(x, y, mask, Wq_w, Wq_b, Wkv_w, Wkv_b, Wo_w)
    nc = _get_program(flags)
    res = bass_utils.run_bass_kernel_spmd(
        nc, in_maps, core_ids=list(range(N_CORES)), trace=trace)
    out = np.empty((B, S, C), dtype=np.float32)
    for b in range(B):
        out[b] = (res.results[2 * b]["out"].astype(np.float32)
                  + res.results[2 * b + 1]["out"].astype(np.float32) + Wo_b)
    return out, res


def kernel(x, y, mask, Wq_w, Wq_b, Wkv_w, Wkv_b, Wo_w, Wo_b):
    out, _ = run(x, y, mask, Wq_w, Wq_b, Wkv_w, Wkv_b, Wo_w, Wo_b, trace=False)
    return out



# revision 50
# speedup vs baseline: 1.1941x; 1.0043x over previous
"""Trainium2 Bass kernel for nn_CausalCrossAttention (B=4, S=2048, C=1024, NH=16, HD=64).

Sharding: DP over batch (4) x TP over heads (2 groups of 8), 8 NeuronCores.
Core c handles batch b = c // 2, head group g = c % 2 (heads 8g..8g+7).

Per-core algorithm (fp16 matmuls everywhere with fp32 PSUM accumulation):
  - Host folds every input to a [128, *] layout with contiguous
    per-partition rows, so each DMA is a single cheap row-block transfer
    (descriptor generation, not bandwidth, limits the startup otherwise);
    weights stream on the scalar queue, activations on the sync queue, both
    in consumption order.  Dummy matmuls on a memset tile warm the PE clock
    (1.2 -> 2.4 GHz) during the ~9us DMA boot window.
  - K-projection -> resident Kt [feat, S]; V-projection -> resident Vx
    [keys, feat] per 128-key tile; Q-projection just-in-time per
    (q-window, pair) via the proj PSUM ring.
  - Scores are computed transposed, S^T[k, q], two heads via PE row-groups
    (0,0)/(64,0) into ONE [128, 1024] two-bank PSUM tile; one exp(s/8)
    activation (strided 3D AP on diagonal tiles skips the dead columns)
    writes P^T (fp16). Causal: skip fully-masked k-tiles, shrink N for
    diagonal tiles, multiply diagonal 128-col blocks by a triangular mask.
  - PV: the two heads run col-tiled (tile_position (0,0)/(0,64)) into one
    [128, 512] PSUM bank, interleaved 2-3 tiles behind the scores so the
    exp pipeline stays covered.
  - Denominators: M=1 matmuls with the (mask) ones-column as lhsT, 4-way
    col-tiled so one pass covers both heads of TWO k-tiles, accumulated in
    a dedicated PSUM bank (partitions 0/32/64/96).
  - Normalize: ONE strided DVE copy stages psD partitions 0..96 in SBUF;
    a single K=97 matmul against a selection constant combines even/odd
    partials AND broadcasts D_A/D_B to the 128 partitions; reciprocal +
    fused multiply write AOt [feat, S] (fp16, O-projection lhsT layout).
  - Unit boundaries are software-pipelined: each unit Q-projects the next
    unit and pre-emits its first two score tiles interleaved with its own
    PV flush, so the PE never waits on the exp backlog at a boundary.
  - O-projection (interleaved after each q-window): out partial
    (AOt.T @ Wo, fp16) staged per 128-row block in one SBUF tile, stored
    fp16 with one DMA (split across both queues for the final two blocks);
    host sums the two head-group partials in fp32 and adds Wo_b.  In the
    last unit, partials for the first TWO trailing blocks run before the
    final normalize (the second borrows a free scores-ring PSUM tile).

PSUM hygiene: pvAB/psD hold multiple independent accumulation chains, and
start=True clears has_written for the WHOLE bank, so both are zero-filled
via DVE memset and all chains accumulate with start=False (measured: the
start=True trick corrupts the sibling chain).  Keep the Pt pool at 5 bufs
(7 measured ~55us slower - SBUF placement is sensitive).
"""

import numpy as np
from contextlib import ExitStack

import concourse.bacc as bacc
import concourse.mybir as mybir
import concourse.tile as tile
from concourse import bass_utils

F32 = mybir.dt.float32
F16 = mybir.dt.float16
AF = mybir.ActivationFunctionType
OP = mybir.AluOpType

B, S, C, NH, HD = 4, 2048, 1024, 16, 64
N_CORES = 8



def build_program(s_len, cin, pairs, hd=64, has_mask=False, has_qkb=False, has_vb=False,
                  disable=frozenset()):
    """Build + compile the per-core Bass program. Returns the Bacc object."""
    assert s_len % 512 == 0 and cin % 128 == 0 and hd == 64
    n_sw = s_len // 512          # 512-wide s windows
    n_ct = cin // 128            # cin contraction tiles
    n_st = s_len // 128          # 128-wide s (key) tiles
    feat = 128 * pairs           # local feature dim (= 64 * local heads)
    n_h = 2 * pairs              # local heads
    cw_sizes = []
    rem = cin
    while rem > 0:
        cw_sizes.append(min(512, rem))
        rem -= 512

    nc = bacc.Bacc("TRN2", target_bir_lowering=False, debug=False,
                   num_devices=N_CORES)

    # all inputs host-folded to [128, *] with contiguous per-partition rows,
    # so every DMA is a single-row-block transfer (cheap descriptor gen)
    d_xT = nc.dram_tensor("xF", [128, n_sw * n_ct * 512], F16,
                          kind="ExternalInput")
    d_yT = nc.dram_tensor("yF", [128, n_sw * n_ct * 512], F16,
                          kind="ExternalInput")
    d_wq = nc.dram_tensor("wqF", [128, n_ct * feat], F16, kind="ExternalInput")
    d_wk = nc.dram_tensor("wkF", [128, n_ct * feat], F16, kind="ExternalInput")
    d_wv = nc.dram_tensor("wvF", [128, n_ct * feat], F16, kind="ExternalInput")
    d_wo = nc.dram_tensor("woF", [128, pairs * cin], F16, kind="ExternalInput")
    d_tri = nc.dram_tensor("tri", [128, 128], F32, kind="ExternalInput")
    d_mvc = nc.dram_tensor("mvc", [128, n_st], F16, kind="ExternalInput")
    d_sel4 = nc.dram_tensor("sel4", [97, 128], F16, kind="ExternalInput")
    if has_qkb:
        d_qb = nc.dram_tensor("qb", [128, pairs], F32, kind="ExternalInput")
        d_kb = nc.dram_tensor("kb", [128, pairs], F32, kind="ExternalInput")
    if has_vb:
        d_vbr = nc.dram_tensor("vbr", [128, feat], F32, kind="ExternalInput")
    if has_mask:
        d_mvst = nc.dram_tensor("mvst", [128, n_st], F32, kind="ExternalInput")
    d_out = nc.dram_tensor("out", [s_len, cin], F16, kind="ExternalOutput")

    with tile.TileContext(nc) as tc, ExitStack() as ctx:
        cpool = ctx.enter_context(tc.tile_pool(name="const", bufs=1))
        wpool = ctx.enter_context(tc.tile_pool(name="wts", bufs=3))
        bigp = ctx.enter_context(tc.tile_pool(name="big", bufs=1))
        sbuf = ctx.enter_context(tc.tile_pool(name="stream", bufs=5))
        qtwp = ctx.enter_context(tc.tile_pool(name="qtw", bufs=3))
        ppool = ctx.enter_context(tc.tile_pool(name="pt", bufs=5))
        rpool = ctx.enter_context(tc.tile_pool(name="rp", bufs=2))
        dpool = ctx.enter_context(tc.tile_pool(name="dp", bufs=2))
        # PSUM budget (8 banks): scores sA 2x[128,1024] = 4, proj ring
        # 2x[128,512] = 2, pvAB 1, psD 1.
        ps_pv = ctx.enter_context(tc.tile_pool(name="ps_pv", bufs=2, space="PSUM"))
        ps_s = ctx.enter_context(tc.tile_pool(name="ps_s", bufs=2, space="PSUM"))

        # --- PE clock warm-up ---------------------------------------------
        # The PE idles at 1.2 GHz until the activity monitor sees ~3.4us of
        # sustained work.  The DMA path only starts moving data ~8.7us in, so
        # run dummy matmuls on a memset tile (DVE boots much earlier) to have
        # the clock warm before the first real matmul.
        wtile = cpool.tile([128, 512], F16, tag="warm")
        nc.vector.memset(wtile[:], 0.0)
        wps = ps_s.tile([128, 1024], F32, tag="sA", name="warm")
        for _ in range(6):
            nc.tensor.matmul(wps[:, 0:512], wtile[:, 0:128], wtile[:],
                             start=True, stop=True)

        # --- input streams (host-folded layouts, two queues) ---------------
        wk = wpool.tile([128, n_ct * feat], F16, tag="w")
        wv = wpool.tile([128, n_ct * feat], F16, tag="w")
        wq = wpool.tile([128, n_ct * feat], F16, tag="w")
        wo = wpool.tile([128, pairs * cin], F16, tag="wo", bufs=1)

        def stream_tile(queue, dst, dsrc, col0, parts):
            # contiguous column-range copies (src layout == tile layout)
            for lo, hi in parts:
                queue.dma_start(dst[:, lo:hi], dsrc[:, col0 + lo:col0 + hi])

        yts_by_w = {}
        xts_by_w = {}
        wcols = n_ct * 512   # columns per s-window in the folded layout

        def emit_y_dmas(sw, parts=None):
            if sw in yts_by_w:
                return
            t = sbuf.tile([128, wcols], F16, tag="s")
            stream_tile(nc.sync, t, d_yT, sw * wcols, parts or [(0, wcols)])
            yts_by_w[sw] = t

        def emit_x_dmas(w, parts=None):
            if w in xts_by_w:
                return
            t = sbuf.tile([128, wcols], F16, tag="s")
            stream_tile(nc.sync, t, d_xT, w * wcols, parts or [(0, wcols)])
            xts_by_w[w] = t

        # weights ride the scalar queue (ScalarE is idle during startup and
        # the PL/gpsimd queue issues descriptors too slowly); activations the
        # sync queue.  Both stream in consumption order so the projection
        # chains can chase the transfers.
        q4 = [(i * wcols // 4, (i + 1) * wcols // 4) for i in range(4)]
        h0 = [(0, wcols // 2), (wcols // 2, wcols)]
        stream_tile(nc.scalar, wk, d_wk, 0, q4)
        emit_y_dmas(0, parts=q4)
        stream_tile(nc.scalar, wq, d_wq, 0, h0)
        emit_x_dmas(0, parts=h0)

        tri = cpool.tile([128, 128], F32, tag="tri")
        nc.sync.dma_start(tri[:], d_tri[:])
        mvc = cpool.tile([128, n_st], F16, tag="mvc")
        nc.sync.dma_start(mvc[:], d_mvc[:])
        sel4 = cpool.tile([97, 128], F16, tag="sel4")
        nc.sync.dma_start(sel4[:], d_sel4[:])
        if has_qkb:
            qb = cpool.tile([128, pairs], F32, tag="qb")
            nc.sync.dma_start(qb[:], d_qb[:])
            kb = cpool.tile([128, pairs], F32, tag="kb")
            nc.sync.dma_start(kb[:], d_kb[:])
        if has_vb:
            vbr = cpool.tile([128, feat], F32, tag="vbr")
            nc.sync.dma_start(vbr[:], d_vbr[:])
        if has_mask:
            mvst = cpool.tile([128, n_st], F32, tag="mvst")
            nc.sync.dma_start(mvst[:], d_mvst[:])

        stream_tile(nc.scalar, wv, d_wv, 0, [(0, n_ct * feat)])
        stream_tile(nc.scalar, wo, d_wo, 0, [(0, pairs * cin)])

        # big persistent tensors
        Kt = bigp.tile([128, pairs * s_len], F16, tag="kt")
        Vx = bigp.tile([128, n_st * feat], F16, tag="vx")
        AOt = bigp.tile([128, pairs * s_len], F16, tag="aot")

        def emit_k_mms(sw):
            yt = yts_by_w[sw]
            for mt in range(pairs):
                pk = ps_pv.tile([128, 512], F32, tag="proj")
                for ct in range(n_ct):
                    nc.tensor.matmul(
                        pk[:],
                        wk[:, ct * feat + mt * 128: ct * feat + (mt + 1) * 128],
                        yt[:, ct * 512:(ct + 1) * 512],
                        start=(ct == 0), stop=(ct == n_ct - 1))
                dst = Kt[:, mt * s_len + sw * 512: mt * s_len + (sw + 1) * 512]
                if has_qkb:
                    nc.vector.tensor_scalar_add(dst, pk[:], kb[:, mt:mt + 1])
                else:
                    nc.vector.tensor_copy(dst, pk[:])

        def emit_v_mms(sw):
            yt = yts_by_w[sw]
            for i in range(4):
                st = sw * 4 + i
                pvp = ps_pv.tile([128, feat], F32, tag="proj")
                for ct in range(n_ct):
                    nc.tensor.matmul(
                        pvp[:],
                        yt[:, ct * 512 + i * 128: ct * 512 + (i + 1) * 128],
                        wv[:, ct * feat:(ct + 1) * feat],
                        start=(ct == 0), stop=(ct == n_ct - 1))
                dst = Vx[:, st * feat:(st + 1) * feat]
                if has_vb:
                    nc.vector.scalar_tensor_tensor(
                        out=dst, in0=pvp[:], scalar=0.0, in1=vbr[:],
                        op0=OP.bypass, op1=OP.add)
                    if has_mask:
                        nc.vector.tensor_scalar_mul(dst, dst, mvst[:, st:st + 1])
                elif has_mask:
                    nc.vector.tensor_scalar_mul(dst, pvp[:], mvst[:, st:st + 1])
                else:
                    nc.vector.tensor_copy(dst, pvp[:])

        qtw_by_unit = {}

        def emit_qproj(w, p):
            xt = xts_by_w[w]
            # pq lives in the proj ring so the scores ring keeps both slots
            # free for the software-pipelined unit boundary
            pq = ps_pv.tile([128, 512], F32, tag="proj")
            for ct in range(n_ct):
                nc.tensor.matmul(
                    pq[:],
                    wq[:, ct * feat + p * 128: ct * feat + (p + 1) * 128],
                    xt[:, ct * 512:(ct + 1) * 512],
                    start=(ct == 0), stop=(ct == n_ct - 1))
            Qtw = qtwp.tile([128, 512], F16, tag="q")
            if has_qkb:
                nc.vector.tensor_scalar_add(Qtw[:], pq[:], qb[:, p:p + 1])
            else:
                # DVE copy: a ScalarE copy here queues behind the exp
                # backlog and stalls the next unit's first scores matmul
                nc.vector.tensor_copy(Qtw[:], pq[:])
            qtw_by_unit[(w, p)] = Qtw

        ot_by_st = {}

        def emit_oproj_part(st, cwi, p4_hi, po=None):
            co = sum(cw_sizes[:cwi])
            cw = cw_sizes[cwi]
            if po is None:
                po = ps_pv.tile([128, 512], F32, tag="proj")
            for p4 in range(p4_hi):
                nc.tensor.matmul(
                    po[:, 0:cw],
                    AOt[:, p4 * s_len + st * 128: p4 * s_len + (st + 1) * 128],
                    wo[:, p4 * cin + co: p4 * cin + co + cw],
                    start=(p4 == 0), stop=(p4 == pairs - 1))
            return po

        def emit_oproj_fin(st, cwi, po, p4_lo):
            co = sum(cw_sizes[:cwi])
            cw = cw_sizes[cwi]
            for p4 in range(p4_lo, pairs):
                nc.tensor.matmul(
                    po[:, 0:cw],
                    AOt[:, p4 * s_len + st * 128: p4 * s_len + (st + 1) * 128],
                    wo[:, p4 * cin + co: p4 * cin + co + cw],
                    start=False, stop=(p4 == pairs - 1))
            # stage all cw blocks of this st in one SBUF tile; single store
            if st in ot_by_st:
                ot = ot_by_st[st]
            else:
                ot = rpool.tile([128, cin], F16, tag="osb", bufs=3)
                ot_by_st[st] = ot
            nc.vector.tensor_copy(ot[:, co:co + cw], po[:, 0:cw])
            done = co + cw == cin
            if st >= n_st - 2:
                # final blocks: store each piece as soon as its copy lands,
                # spread across both DMA queues to shorten the drain
                q = nc.sync if (st + cwi) % 2 else nc.gpsimd
                q.dma_start(d_out[st * 128:(st + 1) * 128, co:co + cw],
                            ot[:, co:co + cw])
                if done:
                    del ot_by_st[st]
            elif done:
                del ot_by_st[st]
                q = nc.sync if st % 2 else nc.gpsimd
                q.dma_start(d_out[st * 128:(st + 1) * 128, :], ot[:])

        def emit_oproj_block(st):
            for cwi in range(len(cw_sizes)):
                po = emit_oproj_part(st, cwi, pairs)
                emit_oproj_fin(st, cwi, po, pairs)

        units = [(w, p) for w in range(n_sw) for p in range(pairs)]
        pre_pend = {}   # ui -> [(kt, c0, Pt)] score tiles pre-emitted by ui-1

        def emit_scores_kt(w, p, Qtw, kt):
            """Scores + exp (+ causal mask) for one 128-key tile."""
            k0 = kt * 128
            diag = kt >= 4 * w
            c0 = (kt - 4 * w) * 128 if diag else 0
            sAB = ps_s.tile([128, 1024], F32, tag="sA")
            nc.tensor.matmul(
                sAB[:, c0:512],
                Kt[0:64, p * s_len + k0: p * s_len + k0 + 128],
                Qtw[0:64, c0:512],
                start=True, stop=True, tile_position=(0, 0))
            nc.tensor.matmul(
                sAB[:, 512 + c0:1024],
                Kt[64:128, p * s_len + k0: p * s_len + k0 + 128],
                Qtw[64:128, c0:512],
                start=True, stop=True, tile_position=(64, 0))
            Pt = ppool.tile([128, 1024], F16, tag="Pt")
            if diag and c0 and "m3ap" not in disable:
                # one activation over both heads' live columns (3D AP
                # skips the dead [512:512+c0] region)
                nc.scalar.activation(
                    Pt.rearrange("p (h q) -> p h q", h=2)[:, :, c0:512],
                    sAB.rearrange("p (h q) -> p h q", h=2)[:, :, c0:512],
                    AF.Exp, scale=0.125)
            else:
                nc.scalar.activation(Pt[:, c0:1024], sAB[:, c0:1024],
                                     AF.Exp, scale=0.125)
            if diag:
                nc.vector.tensor_tensor(
                    out=Pt[:, c0:c0 + 128], in0=Pt[:, c0:c0 + 128],
                    in1=tri[:], op=OP.mult)
                nc.vector.tensor_tensor(
                    out=Pt[:, 512 + c0:512 + c0 + 128],
                    in0=Pt[:, 512 + c0:512 + c0 + 128],
                    in1=tri[:], op=OP.mult)
            return (kt, c0, Pt)

        def emit_unit(ui):
            w, p = units[ui]
            q0 = w * 512
            nxt = units[ui + 1] if ui + 1 < len(units) else None
            # O-projection blocks of the previous window, spread across units
            osts = []
            if w >= 1:
                lo, hi = p * 4 // pairs, (p + 1) * 4 // pairs
                osts = [4 * (w - 1) + i for i in range(lo, hi)]
            Qtw = qtw_by_unit.pop((w, p))
            nkt = 4 * (w + 1)
            n_pass = nkt // 2
            pvAB = ps_pv.tile([128, 512], F32, tag="pv", bufs=1)
            psD = ps_pv.tile([128, 512], F32, tag="psD", bufs=1)
            # Both banks hold multiple independent accumulation chains
            # (col-tiled regions), so no matmul may use the start=True bank
            # clear (measured: the whole-bank bit clear breaks the sibling
            # chain).  Zero-fill instead: accumulate-onto-0 and
            # overwrite-where-clear both yield the right value.
            nc.vector.memset(pvAB[:], 0.0)
            nc.vector.memset(psD[:], 0.0)
            # score tiles pre-emitted in the previous unit's tail
            pend = pre_pend.pop(ui, [])
            start_kt = len(pend)
            dready = []  # PV-emitted k-tiles awaiting a paired D-pass
            npass = [0]

            def emit_pv(pkt, pc0, pPt, stop):
                nc.tensor.matmul(
                    pvAB[0:64, pc0:512],
                    Vx[:, pkt * feat + (2 * p) * 64: pkt * feat + (2 * p) * 64 + 64],
                    pPt[:, pc0:512],
                    start=False, stop=stop, tile_position=(0, 0))
                nc.tensor.matmul(
                    pvAB[64:128, pc0:512],
                    Vx[:, pkt * feat + (2 * p + 1) * 64: pkt * feat + (2 * p + 1) * 64 + 64],
                    pPt[:, 512 + pc0:1024],
                    start=False, stop=stop, tile_position=(0, 64))
                dready.append((pkt, pc0, pPt))
                if len(dready) == 2:
                    emit_dpass(dready[0], dready[1])
                    dready.clear()

            def emit_dpass(d0, d1):
                last = npass[0] == n_pass - 1
                npass[0] += 1
                if "dpass" in disable:
                    return
                for gi, ((kt_, c0_, Pt_), half) in enumerate(
                        [(d0, 0), (d0, 1), (d1, 0), (d1, 1)]):
                    nc.tensor.matmul(
                        psD[32 * gi: 32 * gi + 1, c0_:512],
                        mvc[:, kt_:kt_ + 1],
                        Pt_[:, 512 * half + c0_: 512 * half + 512],
                        start=False,
                        stop=last, tile_position=(0, 32 * gi))

            for kt in range(start_kt, nkt):
                pend.append(emit_scores_kt(w, p, Qtw, kt))
                if kt == 3:
                    for st in osts:
                        emit_oproj_block(st)
                if len(pend) >= 3:
                    emit_pv(*pend.pop(0), stop=False)
            tail_pos = []
            if nxt is None:
                while pend:
                    emit_pv(*pend.pop(0), stop=(not pend))
                # tail O-proj: pair-0..2 partials for the first TWO st blocks
                # of the last window run before the final normalize chain
                # (pair 3's AOt isn't written yet); the second block borrows
                # a free scores-ring PSUM tile.
                st0 = 4 * (n_sw - 1)
                for cwi in range(len(cw_sizes)):
                    tail_pos.append(
                        (st0, cwi, emit_oproj_part(st0, cwi, pairs - 1)))
                po13 = ps_s.tile([128, 1024], F32, tag="sA", name="po13")
                for cwi in range(len(cw_sizes)):
                    tail_pos.append(
                        (st0 + 1, cwi,
                         emit_oproj_part(st0 + 1, cwi, pairs - 1,
                                         po=po13[:, cwi * 512:(cwi + 1) * 512])))
            else:
                # software-pipeline the unit boundary: Q-project the next
                # unit, then interleave its first two score tiles with this
                # unit's PV flush so the PE never waits on the exp backlog.
                emit_qproj(*nxt)
                Qtw_n = qtw_by_unit[nxt]
                wn, pn = nxt
                pend_n = [emit_scores_kt(wn, pn, Qtw_n, 0)]
                emit_pv(*pend.pop(0), stop=(not pend))
                pend_n.append(emit_scores_kt(wn, pn, Qtw_n, 1))
                while pend:
                    emit_pv(*pend.pop(0), stop=(not pend))
                pre_pend[ui + 1] = pend_n
            # normalize: one copy stages psD partitions 0..96 (the four D
            # partial rows live at 0/32/64/96; the rows between hold zeros
            # from the first unit's memset) into SBUF; a single K=97 matmul
            # against a selection constant then combines even/odd partials
            # and broadcasts them to all 128 partitions.
            dsb4 = dpool.tile([97, 512], F16, tag="dsb")
            if "dpass" in disable:
                nc.vector.memset(dsb4[:], 0.25)
            else:
                nc.vector.tensor_copy(dsb4[:], psD[0:97, 0:512])
            rsb = rpool.tile([128, 512], F32, tag="rsb")
            if "bcast" in disable:
                nc.vector.memset(rsb[:], 1.0)
            else:
                if nxt is None:
                    psBf = ps_s.tile([128, 1024], F32, tag="sA", name="psBf")
                    psB = psBf[:, 0:512]
                else:
                    psB = ps_pv.tile([128, 512], F32, tag="proj")
                # psB[pp, q] = D_A[q] for pp<64, D_B[q] for pp>=64
                nc.tensor.matmul(psB[:], sel4[:], dsb4[:],
                                 start=True, stop=True)
                nc.vector.reciprocal_approx_fast(out=rsb[:], in_=psB[:])
            if nxt is None:
                # split the final normalize multiply so the first tail
                # O-projection's weight load can start after 1/4 of it
                for qi in range(4):
                    nc.vector.tensor_tensor(
                        out=AOt[:, p * s_len + q0 + qi * 128:
                                p * s_len + q0 + (qi + 1) * 128],
                        in0=pvAB[:, qi * 128:(qi + 1) * 128],
                        in1=rsb[:, qi * 128:(qi + 1) * 128], op=OP.mult)
            else:
                nc.vector.tensor_tensor(
                    out=AOt[:, p * s_len + q0: p * s_len + q0 + 512],
                    in0=pvAB[:], in1=rsb[:], op=OP.mult)
            for st_, cwi_, po_ in tail_pos:
                emit_oproj_fin(st_, cwi_, po_, pairs - 1)

        # ---- emission schedule ----
        emit_k_mms(0)
        emit_qproj(0, 0)
        emit_v_mms(0)
        for ui in range(len(units)):
            w, p = units[ui]
            if p == 0 and w + 1 < n_sw:
                # prefetch next window's activations at window start
                emit_y_dmas(w + 1)
                emit_x_dmas(w + 1)
            emit_unit(ui)
            # defer window w+1's K/V projections into window w (the late,
            # exp-bound windows then carry more PE filler work)
            if w + 1 < n_sw:
                if p == 0:
                    emit_k_mms(w + 1)
                elif p == 1:
                    emit_v_mms(w + 1)
        for st in range(4 * (n_sw - 1) + 2, 4 * n_sw):   # last window's O-proj
            emit_oproj_block(st)

    nc.compile()
    return nc


_programs = {}


def _get_program(key):
    if key not in _programs:
        _programs[key] = build_program(S, C, 4, HD, *key)
    return _programs[key]


def make_core_inputs(x, y, mask, Wq_w, Wq_b, Wkv_w, Wkv_b, Wo_w,
                     s_len=S, cin=C, pairs=4, nh=NH):
    """Build the list of 8 per-core input dicts (host-side shard + permute)."""
    n_h = 2 * pairs
    feat = 128 * pairs
    has_mask = bool(np.any(mask))
    has_qkb = bool(np.any(Wq_b)) or bool(np.any(Wkv_b[:cin]))
    has_vb = bool(np.any(Wkv_b[cin:]))
    tri = np.triu(np.ones((128, 128), dtype=np.float32))
    n_st = s_len // 128

    in_maps = []
    for core in range(N_CORES):
        b = core // 2
        g = core % 2
        # feature permutation: col = 128*p + 64*half + d  <-  local head 2p+half
        cidx = np.arange(feat)
        pair_i = cidx // 128
        half = (cidx % 128) // 64
        d = cidx % 64
        qk_rows = (n_h * g + 2 * pair_i + half) * 64 + d
        v_rows = cin + (n_h * g + cidx // 64) * 64 + cidx % 64

        mvec = 1.0 - mask[b].astype(np.float32)
        sel4 = np.zeros((97, 128), dtype=np.float16)
        sel4[0, 0:64] = 1.0    # head-A even-k-tile partials
        sel4[32, 64:128] = 1.0  # head-B even
        sel4[64, 0:64] = 1.0   # head-A odd
        sel4[96, 64:128] = 1.0  # head-B odd
        n_ct = cin // 128
        n_sw = s_len // 512

        def fold(a):
            # [n*128, m] -> [128, n*m] with [p, i*m + j] = a[i*128+p, j]
            n = a.shape[0] // 128
            return np.ascontiguousarray(
                a.reshape(n, 128, a.shape[1]).transpose(1, 0, 2)
                .reshape(128, n * a.shape[1]))

        def fold_act(aT):
            # [cin, s_len] -> [128, n_sw * n_ct * 512] window-major:
            # [p, w*(n_ct*512) + ct*512 + j] = aT[ct*128+p, w*512+j]
            return np.ascontiguousarray(
                aT.reshape(n_ct, 128, n_sw, 512).transpose(1, 2, 0, 3)
                .reshape(128, n_sw * n_ct * 512))

        m = {
            "xF": fold_act(x[b].T.astype(np.float16)),
            "yF": fold_act(y[b].T.astype(np.float16)),
            "wqF": fold(Wq_w[qk_rows, :].T.astype(np.float16)),
            "wkF": fold(Wkv_w[qk_rows, :].T.astype(np.float16)),
            "wvF": fold(Wkv_w[v_rows, :].T.astype(np.float16)),
            "woF": fold(Wo_w[:, qk_rows].T.astype(np.float16)),
            "tri": tri,
            "mvc": np.ascontiguousarray(
                mvec.reshape(n_st, 128).T).astype(np.float16),
            "sel4": sel4,
        }
        if has_qkb:
            m["qb"] = np.ascontiguousarray(
                Wq_b[qk_rows].reshape(pairs, 128).T)
            m["kb"] = np.ascontiguousarray(
                Wkv_b[qk_rows].reshape(pairs, 128).T)
        if has_vb:
            m["vbr"] = np.tile(Wkv_b[v_rows][None, :], (128, 1))
        if has_mask:
            m["mvst"] = np.ascontiguousarray(
                mvec.reshape(n_st, 128).T)
        in_maps.append(m)
    return in_maps, (has_mask, has_qkb, has_vb)


def run(x, y, mask, Wq_w, Wq_b, Wkv_w, Wkv_b, Wo_w, Wo_b, trace=False):
    x = np.asarray(x, dtype=np.float32)
    y = np.asarray(y, dtype=np.float32)
    mask = np.asarray(mask)
    Wq_w = np.asarray(Wq_w, dtype=np.float32)
    Wq_b = np.asarray(Wq_b, dtype=np.float32)
    Wkv_w = np.asarray(Wkv_w, dtype=np.float32)
    Wkv_b = np.asarray(Wkv_b, dtype=np.float32)
    Wo_w = np.asarray(Wo_w, dtype=np.float32)
    Wo_b = np.asarray(Wo_b, dtype=np.float32)

    in_maps, flags = make_core_inputs(x, y, mask, Wq_w, Wq_b, Wkv_w, Wkv_b, Wo_w)
    nc = _get_program(flags)
    res = bass_utils.run_bass_kernel_spmd(
        nc, in_maps, core_ids=list(range(N_CORES)), trace=trace)
    out = np.empty((B, S, C), dtype=np.float32)
    for b in range(B):
        out[b] = (res.results[2 * b]["out"].astype(np.float32)
                  + res.results[2 * b + 1]["out"].astype(np.float32) + Wo_b)
    return out, res


def kernel(x, y, mask, Wq_w, Wq_b, Wkv_w, Wkv_b, Wo_w, Wo_b):
    out, _ = run(x, y, mask, Wq_w, Wq_b, Wkv_w, Wkv_b, Wo_w, Wo_b, trace=False)
    return out

